# revision 1
# baseline (speedup 1.0000x reference)
"""MAB (Set-Transformer multihead attention block) Trainium2 Bass kernel.

Reference math (fp32):
  Q = q @ Wq.T + bq ; K = k @ Wk.T + bk ; V = k @ Wv.T + bv    [B,N,256]
  per head h (8 heads x 32): s = Qh @ Kh.T / 16 ; a = softmax(s)
  Oh = Qh + a @ Vh ; o = concat(Oh) ; o = LN0(o) ; o = o + relu(o @ Wo.T + bo)
  out = LN1(o)

Sharding: 8 cores = (batch b in 0..3, query-half in 0..1). Each core handles
1024 queries x 2048 keys of one batch; rows are fully independent through
the whole block (LN/FFN are per-row), so there are no collectives.

Per-core pipeline (all on-chip, "transposed" layout = feature dim on
partitions, tokens on the free dim):
  - PE-transpose q, k, W* (fp32 has no DMA transpose)
  - fp32 projections Q_T/K_T/V; V stored per-head-augmented bf16
  - Q/K replicated to all four 32-row partition groups (PE matmul with a
    block-identity) so score matmuls can use 4x row-tiled (tile_position)
    concurrent K=32 matmuls
  - exp on ACT in [128, 8*256] batches straight out of PSUM (ACT is the
    bottleneck engine: 16.8M exps/core at 1 elem/lane/cycle)
  - PV and softmax-denominator via 4x col-tiled matmuls (M=32 heads), with
    denominator = ones-matmul over the same bf16 attn tile
  - normalize+residual on DVE (vector reciprocal, no ACT recip)
  - LN stats via ones-vector matmuls (partition reduction) + K=1 broadcast
    matmuls; rstd = exp(-0.5*ln(var+eps)) keeps ACT on one table set
  - FFN with relu+bias fused into one DVE scalar_tensor_tensor
  - PE-transpose back to natural layout and DMA out
"""

import os
import sys
from contextlib import ExitStack

import numpy as np

for _p in ("/opt/trn_rl_repo", "/root/.axon_site/_ro/trn_rl_repo"):
    if os.path.isdir(_p) and _p not in sys.path:
        sys.path.insert(0, _p)

import concourse.bass as bass  # noqa: E402
import concourse.tile as tile  # noqa: E402
from concourse import bacc, mybir  # noqa: E402
from concourse.masks import make_identity  # noqa: E402

F32 = mybir.dt.float32
BF16 = mybir.dt.bfloat16
P = 128
EPS = 1e-5

AF = mybir.ActivationFunctionType
OP = mybir.AluOpType


class Cfg:
    def __init__(self, NQ=1024, NK=2048, D=256, H=8, SC=256, LC=512):
        self.NQ, self.NK, self.D, self.H = NQ, NK, D, H
        self.HD = D // H            # 32
        self.DO = D // P            # 2
        self.QT = NQ // P
        self.KT = NK // P
        self.SC = min(SC, NQ)       # score/attn q-chunk
        self.LC = min(LC, NQ)       # ln/ffn q-chunk
        self.QCN = NQ // self.SC
        self.LCN = NQ // self.LC
        assert self.HD == 32 and self.DO == 2
        assert self.KT % 8 == 0 or self.KT == 4


def _emit(nc: bass.Bass, tc: tile.TileContext, ctx: ExitStack, io: dict, cfg: Cfg):
    NQ, NK, D, H = cfg.NQ, cfg.NK, cfg.D, cfg.H
    DO, QT, KT, SC, LC = cfg.DO, cfg.QT, cfg.KT, cfg.SC, cfg.LC
    KG = 8 if KT % 8 == 0 else 4    # kt tiles per exp group
    G2 = KG // 4

    const = ctx.enter_context(tc.tile_pool(name="const", bufs=1))
    persist = ctx.enter_context(tc.tile_pool(name="persist", bufs=1))
    big = ctx.enter_context(tc.tile_pool(name="big", bufs=3))
    tmp = ctx.enter_context(tc.tile_pool(name="tmp", bufs=2))

    # ---- constants ----
    ident = const.tile([P, P], F32)
    make_identity(nc, ident)
    rep_full = const.tile([P, P], F32)  # 32x32 identity in every block
    nc.gpsimd.memset(rep_full, 0.0)
    for i in range(4):
        for j in range(4):
            make_identity(nc, rep_full[32 * i:32 * i + 32, 32 * j:32 * j + 32],
                          nomemset=True)
    ones_k = const.tile([P, 1], F32)    # lhsT: partition-sum (K=128, M=1)
    nc.vector.memset(ones_k, 1.0)
    ones32 = const.tile([P, 32], BF16)  # lhsT: softmax denominator
    nc.vector.memset(ones32, 1.0)
    zerot = const.tile([P, 1], F32)
    nc.vector.memset(zerot, 0.0)
    eps_t = const.tile([P, 1], F32)
    nc.vector.memset(eps_t, EPS)

    # ---- small vectors: partition-major [P, DO] layout v[p,o] = vec[o*128+p]
    def vec_pm(name):
        t = const.tile([P, DO], F32, name=f"{name}_pm")
        nc.sync.dma_start(t, io[name][:].rearrange("(o p) -> p o", p=P))
        return t

    bq_pm, bk_pm, bo_pm = vec_pm("bq"), vec_pm("bk"), vec_pm("bo")
    g0_pm, b0_pm = vec_pm("g0"), vec_pm("b0")
    g1_pm, b1_pm = vec_pm("g1"), vec_pm("b1")

    # ---- weights: W^T in sbuf as [pi, po, c]  (= W^T[po*128+pi, c]) ----
    with tc.tile_pool(name="ph0psum", bufs=1, space="PSUM") as psT, \
         tc.tile_pool(name="ph0", bufs=1) as ph0:

        def load_wT(name):
            w_sb = ph0.tile([P, DO, D], F32, name=f"{name}_nat", tag=f"wl{name}")
            nc.sync.dma_start(w_sb, io[name][:].rearrange("(o p) f -> p o f", p=P))
            wT = persist.tile([P, DO, D], F32, name=f"{name}T")
            for o in range(DO):
                for fo in range(DO):
                    ps = psT.tile([P, P], F32, tag="tps", bufs=2)
                    nc.tensor.transpose(ps, w_sb[:, o, fo * P:(fo + 1) * P], ident)
                    nc.vector.tensor_copy(wT[:, fo, o * P:(o + 1) * P], ps)
            return wT

        wqT, wkT, wvT, woT = (load_wT(n) for n in ("Wq", "Wk", "Wv", "Wo"))

        # bv broadcast across partitions: B_v[p, d] = bv[d]
        bv_row = ph0.tile([1, D], F32, name="bv_row")
        nc.sync.dma_start(bv_row, io["bv"][:].rearrange("(o d) -> o d", o=1))
        B_v = persist.tile([P, D], F32, name="B_v")
        nc.gpsimd.partition_broadcast(B_v, bv_row)

        # ---- transpose inputs: q_T [P, DO, NQ], k_T [P, DO, NK] (fp32) ----
        q_sb = ph0.tile([P, QT, D], F32, name="q_nat")
        nc.sync.dma_start(q_sb, io["q"][:].rearrange("(t p) d -> p t d", p=P))
        k_sb = ph0.tile([P, KT, D], F32, name="k_nat")
        nc.sync.dma_start(k_sb, io["k"][:].rearrange("(t p) d -> p t d", p=P))

        q_T = ph0.tile([P, DO, NQ], F32, name="q_T")
        k_T = ph0.tile([P, DO, NK], F32, name="k_T")
        for (src, dst, T) in ((q_sb, q_T, QT), (k_sb, k_T, KT)):
            for t in range(T):
                for o in range(DO):
                    ps = psT.tile([P, P], F32, tag="tps", bufs=2)
                    nc.tensor.transpose(ps, src[:, t, o * P:(o + 1) * P], ident)
                    nc.vector.tensor_copy(dst[:, o, t * P:(t + 1) * P], ps)

        # ---- projections (fp32) ----
        Q_T = persist.tile([P, DO, NQ], F32, name="Q_T")
        K_T = ph0.tile([P, DO, NK], F32, name="K_T")
        with tc.tile_pool(name="ph1psum", bufs=1, space="PSUM") as psA:
            for (wT, src, dst, b_pm, N) in (
                (wqT, q_T, Q_T, bq_pm, NQ),
                (wkT, k_T, K_T, bk_pm, NK),
            ):
                for o in range(DO):
                    for c0 in range(0, N, 512):
                        w = min(512, N - c0)
                        ps = psA.tile([P, 512], F32, tag="proj", bufs=2)
                        for ki in range(DO):
                            nc.tensor.matmul(
                                ps[:, :w], lhsT=wT[:, ki, o * P:(o + 1) * P],
                                rhs=src[:, ki, c0:c0 + w],
                                start=(ki == 0), stop=(ki == DO - 1))
                        nc.vector.tensor_scalar_add(
                            dst[:, o, c0:c0 + w], ps[:, :w], b_pm[:, o:o + 1])

            # V augmented per head: [P, KT, H, 33] bf16, col 32 = 1.0
            V_aug = persist.tile([P, KT, H, 33], BF16, name="V_aug")
            nc.vector.memset(V_aug[:, :, :, 32], 1.0)
            for t in range(KT):
                ps = psA.tile([P, D], F32, tag="vproj", bufs=1)
                for ki in range(DO):
                    nc.tensor.matmul(
                        ps, lhsT=k_T[:, ki, t * P:(t + 1) * P], rhs=wvT[:, ki, :],
                        start=(ki == 0), stop=(ki == DO - 1))
                nc.vector.tensor_tensor(
                    V_aug[:, t, :, :32],
                    ps.rearrange("p (h w) -> p h w", h=H),
                    B_v.rearrange("p (h w) -> p h w", h=H), OP.add)

            # replicate Q/K head rows into all 4 partition groups (bf16)
            Q_rep = persist.tile([P, H, NQ], BF16, name="Q_rep")
            K_rep = persist.tile([P, H, NK], BF16, name="K_rep")
            for (src, dst, N) in ((Q_T, Q_rep, NQ), (K_T, K_rep, NK)):
                for h in range(H):
                    m = h % 4
                    for c0 in range(0, N, 512):
                        w = min(512, N - c0)
                        ps = psA.tile([P, 512], F32, tag="rep", bufs=2)
                        nc.tensor.matmul(
                            ps[:, :w], lhsT=rep_full[32 * m:32 * m + 32, :],
                            rhs=src[32 * m:32 * m + 32, h // 4, c0:c0 + w],
                            start=True, stop=True, tile_position=(32 * m, 0))
                        nc.vector.tensor_copy(dst[:, h, c0:c0 + w], ps[:, :w])

    # ---- attention ----
    O_T = big.tile([P, DO, NQ], F32, name="O_T", tag="big")
    with tc.tile_pool(name="attn", bufs=2) as attn_pool, \
         tc.tile_pool(name="ps_s", bufs=1, space="PSUM") as ps_s, \
         tc.tile_pool(name="ps_o", bufs=2, space="PSUM") as ps_o, \
         tc.tile_pool(name="ps_d", bufs=2, space="PSUM") as ps_d, \
         tc.tile_pool(name="rec_p", bufs=2) as rec_p:
        for hg in range(2):
            for qc in range(cfg.QCN):
                attn = []
                for m in range(4):
                    h = hg * 4 + m
                    at = attn_pool.tile([P, KT, SC], BF16, tag=f"attn{m}")
                    attn.append(at)
                    for g in range(KT // KG):
                        ps = ps_s.tile([P, KG, SC], F32, tag="s")
                        for j in range(KG):
                            kt = g * KG + j
                            # K_rep/Q_rep hold 4 identical head replicas on the
                            # partition dim, so a full K=128 contraction gives
                            # exactly 4x the per-head score -> scale 1/(16*4).
                            nc.tensor.matmul(
                                ps[:, j, :],
                                lhsT=K_rep[:, h, kt * P:(kt + 1) * P],
                                rhs=Q_rep[:, h, qc * SC:(qc + 1) * SC],
                                start=True, stop=True)
                        nc.scalar.activation(at[:, g * KG:(g + 1) * KG, :], ps,
                                             AF.Exp, scale=1.0 / 64.0)
                po = ps_o.tile([P, SC], F32, tag="o")
                pd = ps_d.tile([P, SC], F32, tag="d")
                for t in range(KT):
                    for m in range(4):
                        h = hg * 4 + m
                        nc.tensor.matmul(
                            po[32 * m:32 * m + 32, :], lhsT=V_aug[:, t, h, :32],
                            rhs=attn[m][:, t, :],
                            start=(t == 0), stop=(t == KT - 1),
                            tile_position=(0, 32 * m), skip_group_check=True)
                        nc.tensor.matmul(
                            pd[32 * m:32 * m + 32, :], lhsT=ones32,
                            rhs=attn[m][:, t, :],
                            start=(t == 0), stop=(t == KT - 1),
                            tile_position=(0, 32 * m), skip_group_check=True)
                rec = rec_p.tile([P, SC], F32, tag="rec")
                nc.vector.reciprocal(rec, pd)
                sl = O_T[:, hg, qc * SC:(qc + 1) * SC]
                nc.vector.tensor_tensor(sl, po, rec, OP.mult)
                nc.vector.tensor_add(sl, sl, Q_T[:, hg, qc * SC:(qc + 1) * SC])

    # ---- layernorm helper (transposed layout) ----
    def layer_norm(src, dst, g_pm, b_pm):
        with tc.tile_pool(name="ln_ps", bufs=1, space="PSUM") as lps, \
             tc.tile_pool(name="ln_sb", bufs=2) as lsb:
            for c in range(cfg.LCN):
                csl = slice(c * LC, (c + 1) * LC)
                x2 = lsb.tile([P, DO, LC], F32, tag="x2")
                for o in range(DO):
                    nc.vector.tensor_tensor(
                        x2[:, o, :], src[:, o, csl], src[:, o, csl], OP.mult)
                p_sx = lps.tile([1, LC], F32, tag="sx", bufs=2)
                p_sx2 = lps.tile([1, LC], F32, tag="sx2", bufs=2)
                for o in range(DO):
                    nc.tensor.matmul(p_sx, lhsT=ones_k, rhs=src[:, o, csl],
                                     start=(o == 0), stop=(o == DO - 1))
                    nc.tensor.matmul(p_sx2, lhsT=ones_k, rhs=x2[:, o, :],
                                     start=(o == 0), stop=(o == DO - 1))
                mu = lsb.tile([1, LC], F32, tag="mu")
                var = lsb.tile([1, LC], F32, tag="var")
                A = lsb.tile([1, LC], F32, tag="A")
                Bt = lsb.tile([1, LC], F32, tag="B")
                nc.vector.tensor_scalar_mul(mu, p_sx, 1.0 / D)
                nc.vector.tensor_scalar_mul(var, p_sx2, 1.0 / D)
                nc.vector.tensor_tensor(A, mu, mu, OP.mult)
                nc.vector.tensor_tensor(var, var, A, OP.subtract)
                # rstd = exp(-0.5 * ln(var + eps)); stays on the exp/ln table set
                nc.scalar.activation(A, var, AF.Ln, bias=eps_t[:1])
                nc.scalar.activation(A, A, AF.Exp, scale=-0.5)
                nc.vector.scalar_tensor_tensor(
                    Bt, in0=mu, scalar=-1.0, in1=A, op0=OP.mult, op1=OP.mult)
                pA = lsb.tile([P, LC], F32, tag="bcA")
                pB = lsb.tile([P, LC], F32, tag="bcB")
                nc.gpsimd.partition_broadcast(pA, A)
                nc.gpsimd.partition_broadcast(pB, Bt)
                for o in range(DO):
                    d = dst[:, o, csl]
                    nc.vector.tensor_tensor(d, src[:, o, csl], pA, OP.mult)
                    nc.vector.tensor_tensor(d, d, pB, OP.add)
                    nc.vector.scalar_tensor_tensor(
                        d, in0=d, scalar=g_pm[:, o:o + 1],
                        in1=b_pm[:, o:o + 1].to_broadcast([P, LC]),
                        op0=OP.mult, op1=OP.add)

    X0 = big.tile([P, DO, NQ], F32, name="X0", tag="big")
    layer_norm(O_T, X0, g0_pm, b0_pm)

    # ---- FFN: X1 = X0 + relu(Wo-matmul(X0) + bo) ----
    X1 = big.tile([P, DO, NQ], F32, name="X1", tag="big")
    with tc.tile_pool(name="ffn_ps", bufs=2, space="PSUM") as fps, \
         tc.tile_pool(name="ffn_sb", bufs=2) as fsb:
        for o in range(DO):
            for c in range(cfg.LCN):
                csl = slice(c * LC, (c + 1) * LC)
                ps = fps.tile([P, LC], F32, tag="ffn")
                for ki in range(DO):
                    nc.tensor.matmul(ps, lhsT=woT[:, ki, o * P:(o + 1) * P],
                                     rhs=X0[:, ki, csl],
                                     start=(ki == 0), stop=(ki == DO - 1))
                ht = fsb.tile([P, LC], F32, tag="ht")
                nc.vector.scalar_tensor_tensor(
                    ht, in0=ps, scalar=bo_pm[:, o:o + 1],
                    in1=zerot.to_broadcast([P, LC]), op0=OP.add, op1=OP.max)
                nc.vector.tensor_add(X1[:, o, csl], X0[:, o, csl], ht)

    X2 = big.tile([P, DO, NQ], F32, name="X2", tag="big")
    layer_norm(X1, X2, g1_pm, b1_pm)

    # ---- transpose back to natural [tok, D] and store ----
    with tc.tile_pool(name="out_ps", bufs=4, space="PSUM") as ops, \
         tc.tile_pool(name="out_sb", bufs=1) as osb:
        out_sb = osb.tile([P, QT, D], F32, name="out_nat")
        for t in range(QT):
            for o in range(DO):
                ps = ops.tile([P, P], F32, tag="ops")
                nc.tensor.transpose(ps, X2[:, o, t * P:(t + 1) * P], ident)
                nc.vector.tensor_copy(out_sb[:, t, o * P:(o + 1) * P], ps)
        nc.sync.dma_start(io["out"][:].rearrange("(t p) d -> p t d", p=P), out_sb)


def build(cfg: Cfg) -> bass.Bass:
    nc = bacc.Bacc("TRN2")
    io = {}
    for name, shape in (
        ("q", [cfg.NQ, cfg.D]), ("k", [cfg.NK, cfg.D]),
        ("Wq", [cfg.D, cfg.D]), ("Wk", [cfg.D, cfg.D]),
        ("Wv", [cfg.D, cfg.D]), ("Wo", [cfg.D, cfg.D]),
        ("bq", [cfg.D]), ("bk", [cfg.D]), ("bv", [cfg.D]), ("bo", [cfg.D]),
        ("g0", [cfg.D]), ("b0", [cfg.D]), ("g1", [cfg.D]), ("b1", [cfg.D]),
    ):
        io[name] = nc.dram_tensor(name, shape, F32, kind="ExternalInput")
    io["out"] = nc.dram_tensor("out", [cfg.NQ, cfg.D], F32, kind="ExternalOutput")

    with tile.TileContext(nc) as tc:
        with ExitStack() as ctx:
            _emit(nc, tc, ctx, io, cfg)
    nc.compile()
    return nc


_CACHE = {}


def _get_nc(key, cfg):
    if key not in _CACHE:
        _CACHE[key] = build(cfg)
    return _CACHE[key]


def kernel(q, k, Wq, bq, Wk, bk, Wv, bv, Wo, bo, g0, b0, g1, b1, _trace=False):
    from concourse.bass_utils import run_bass_kernel_spmd

    B, Nq, D = q.shape
    Nk = k.shape[1]
    n_cores = 8
    halves = n_cores // B
    nq_c = Nq // halves
    cfg = Cfg(NQ=nq_c, NK=Nk, D=D)
    nc = _get_nc((nq_c, Nk, D), cfg)

    shared = dict(Wq=Wq, bq=bq, Wk=Wk, bk=bk, Wv=Wv, bv=bv, Wo=Wo, bo=bo,
                  g0=g0, b0=b0, g1=g1, b1=b1)
    shared = {n: np.ascontiguousarray(v, dtype=np.float32)
              for n, v in shared.items()}
    in_maps = []
    for c in range(n_cores):
        b, hf = c // halves, c % halves
        m = dict(shared)
        m["q"] = np.ascontiguousarray(q[b, hf * nq_c:(hf + 1) * nq_c], np.float32)
        m["k"] = np.ascontiguousarray(k[b], np.float32)
        in_maps.append(m)

    res = run_bass_kernel_spmd(nc, in_maps, core_ids=list(range(n_cores)),
                               trace=_trace)
    out = np.empty((B, Nq, D), np.float32)
    for c in range(n_cores):
        b, hf = c // halves, c % halves
        out[b, hf * nq_c:(hf + 1) * nq_c] = res.results[c]["out"]
    if _trace:
        return out, res
    return out



# revision 12
# speedup vs baseline: 1.4708x; 1.4708x over previous
"""MAB (Set-Transformer multihead attention block) Trainium2 Bass kernel, v2.

Reference math (fp32):
  Q = q @ Wq.T + bq ; K = k @ Wk.T + bk ; V = k @ Wv.T + bv    [B,N,256]
  per head h (8 heads x 32): s = Qh @ Kh.T / 16 ; a = softmax(s)
  Oh = Qh + a @ Vh ; o = concat(Oh) ; o = LN0(o) ; o = o + relu(o @ Wo.T + bo)
  out = LN1(o)

Sharding: 8 cores = (batch b in 0..3, query-half in 0..1); no collectives.

v2 design vs v1:
  - scores: 4-way row-tiled concurrent K=32 matmuls (one per head of an
    o-block) straight off the natural head layout -- no Q/K replication.
  - exp split between ACT (table exp) and DVE (Schraudolph-style bf16
    bit-trick via tensor_scalar f32->i16 with rounding, ~+-3.3% per
    element, irrelevant after softmax averaging over 2048 keys).
  - all linear-layer biases folded into K=1 ones-row matmuls so every
    PSUM drain is a plain copy, assignable to either ACT or DVE.
  - softmax denominators via ones32 matmuls interleaved with PV per key
    tile (attn tiles freed incrementally; exp never stalls on PV).
  - 1/d via RECIPROCAL_APPROX_FAST (1 DVE inst, ~51 ULP).
  - LN: stats via ones matmuls; row math done in a [128, chunks] layout
    (PE piece-transposes) so DVE works 128 lanes wide; rstd via
    reciprocal_approx_fast + sqrt bit-trick + 1 Newton step => no Ln/Sqrt
    ACT tables (single act table load for the whole kernel).
  - LN1 apply fused into the final transpose drains (per-partition
    scalars after transpose).
"""

import os
import sys
from contextlib import ExitStack

import numpy as np

for _p in ("/opt/trn_rl_repo", "/root/.axon_site/_ro/trn_rl_repo"):
    if os.path.isdir(_p) and _p not in sys.path:
        sys.path.insert(0, _p)

import concourse.bass as bass  # noqa: E402
import concourse.tile as tile  # noqa: E402
from concourse import bacc, mybir  # noqa: E402
from concourse.masks import make_identity  # noqa: E402

F32 = mybir.dt.float32
BF16 = mybir.dt.bfloat16
I16 = mybir.dt.int16
I32 = mybir.dt.int32
P = 128
EPS = 1e-5

AF = mybir.ActivationFunctionType
OP = mybir.AluOpType

LOG2E = 1.4426950408889634
# exp(x/16) ~= bf16(bits = round(x*EXP_A + EXP_B))
EXP_A = 128.0 * LOG2E / 16.0
EXP_B = 128.0 * (127.0 - 0.04380)
# rsqrt(v) ~= bf16(bits = round(bits32(v)*RS_A + RS_B)), then 1 Newton step
RS_A = -64.0 / (1 << 23)
RS_B = 24375.283445

ACT_EXP_SHARE = 0.60  # fraction of exp drains on ScalarE


class Cfg:
    def __init__(self, NQ=1024, NK=2048, D=256, H=8, ln0_gb=False, ln1_gb=False):
        self.NQ, self.NK, self.D, self.H = NQ, NK, D, H
        self.HD = D // H            # 32
        self.DO = D // P            # 2
        self.QT = NQ // P           # 8
        self.KT = NK // P           # 16
        self.SC = 512               # score/attn q-chunk
        self.QCN = NQ // self.SC    # 2
        self.LC = 512               # ln/ffn q-chunk
        self.LCN = NQ // self.LC
        self.ln0_gb = ln0_gb        # apply non-trivial g0/b0
        self.ln1_gb = ln1_gb
        assert self.HD == 32 and self.DO == 2


class EngineSplit:
    """Bresenham-style assigner: returns True for ACT with given share."""

    def __init__(self, share):
        self.share = share
        self.acc = 0.0

    def take(self):
        self.acc += self.share
        if self.acc >= 1.0:
            self.acc -= 1.0
            return True
        return False


def _emit(nc: bass.Bass, tc: tile.TileContext, ctx: ExitStack, io: dict, cfg: Cfg):
    NQ, NK, D, H = cfg.NQ, cfg.NK, cfg.D, cfg.H
    DO, QT, KT, SC, LC = cfg.DO, cfg.QT, cfg.KT, cfg.SC, cfg.LC

    const = ctx.enter_context(tc.tile_pool(name="const", bufs=1))
    persist = ctx.enter_context(tc.tile_pool(name="persist", bufs=1))

    # ---- constants ----
    ident = const.tile([P, P], F32)
    make_identity(nc, ident)
    ones32 = const.tile([P, 32], BF16)
    nc.vector.memset(ones32, 1.0)
    ones_r = const.tile([1, 512], F32)   # ones row: bias-matmul rhs / lhsT
    nc.vector.memset(ones_r, 1.0)
    ones_k = const.tile([P, 1], F32)     # LN stats lhsT (partition sum)
    nc.vector.memset(ones_k, 1.0)

    def vec_row(name):
        t = const.tile([1, D], F32, name=f"{name}_row")
        nc.sync.dma_start(t, io[name][:].rearrange("(o d) -> o d", o=1))
        return t

    bq_row, bk_row, bv_row = vec_row("bq"), vec_row("bk"), vec_row("bv")

    def vec_pm(name):
        t = const.tile([P, DO], F32, name=f"{name}_pm")
        nc.sync.dma_start(t, io[name][:].rearrange("(o p) -> p o", p=P))
        return t

    bo_pm = vec_pm("bo")
    g0_pm = vec_pm("g0") if cfg.ln0_gb else None
    b0_pm = vec_pm("b0") if cfg.ln0_gb else None
    # g1/b1 broadcast along features (free dim after final transpose)
    if cfg.ln1_gb:
        g1_row = vec_row("g1")
        b1_row = vec_row("b1")
        g1_bc = const.tile([P, D], F32)
        nc.gpsimd.partition_broadcast(g1_bc, g1_row)
        b1_bc = const.tile([P, D], F32)
        nc.gpsimd.partition_broadcast(b1_bc, b1_row)

    # ---- persistent tensors ----
    Q_bf = persist.tile([P, DO, NQ], BF16, name="Q_bf")
    K_bf = persist.tile([P, DO, NK], BF16, name="K_bf")
    V_nat = persist.tile([P, KT, H, 32], BF16, name="V_nat")  # [tok, dv] per tile
    woT = persist.tile([P, DO, D], F32, name="woT")
    O_T = persist.tile([P, DO, NQ], F32, name="O_T")
    X0 = persist.tile([P, DO, NQ], F32, name="X0")
    X1 = persist.tile([P, DO, NQ], F32, name="X1")
    out_nat = persist.tile([P, QT, D], F32, name="out_nat")

    drain_split = EngineSplit(0.5)

    def drain(dst, src):
        if drain_split.take():
            nc.scalar.copy(dst, src)
        else:
            nc.vector.tensor_copy(dst, src)

    # =================== prologue: transposes + projections ===================
    with tc.tile_pool(name="ph0", bufs=1) as ph0, \
         tc.tile_pool(name="tps", bufs=1, space="PSUM") as tps, \
         tc.tile_pool(name="pps", bufs=1, space="PSUM") as pps:

        k_sb = ph0.tile([P, KT, D], F32, name="k_sb")
        nc.sync.dma_start(k_sb, io["k"][:].rearrange("(t p) d -> p t d", p=P))
        q_sb = ph0.tile([P, QT, D], F32, name="q_sb")
        nc.sync.dma_start(q_sb, io["q"][:].rearrange("(t p) d -> p t d", p=P))

        tp_rot = [0]

        def transpose4(pieces, dst):
            """PE-transpose four [128,128] pieces into one psum bank, one
            drain into dst (a [P, 2, 2, 128]-shaped AP)."""
            ps = tps.tile([P, 4, P], F32, tag=f"tp{tp_rot[0] % 3}", name="tp")
            tp_rot[0] += 1
            for j, src in enumerate(pieces):
                nc.tensor.transpose(ps[:, j, :], src, ident)
            drain(dst, ps[:].rearrange("p (a b) c -> p a b c", a=2))

        def load_wT(name, dst):
            w_sb = ph0.tile([P, DO, D], F32, name=f"{name}_nat", tag="wl")
            nc.sync.dma_start(w_sb, io[name][:].rearrange("(o p) f -> p o f", p=P))
            # dst[p, fo, o*128+c] = W^T block; piece order (fo, o)
            transpose4(
                [w_sb[:, o, fo * P:(fo + 1) * P] for fo in range(2) for o in range(2)],
                dst[:].rearrange("p f (o c) -> p f o c", c=P))
            return dst

        wkT = load_wT("Wk", ph0.tile([P, DO, D], F32, name="wkT"))
        wvT = load_wT("Wv", ph0.tile([P, DO, D], F32, name="wvT"))
        wqT = load_wT("Wq", ph0.tile([P, DO, D], F32, name="wqT"))
        load_wT("Wo", woT)

        k_T = ph0.tile([P, DO, NK], F32, name="k_T")
        for t0 in range(0, KT, 2):
            transpose4(
                [k_sb[:, t, o * P:(o + 1) * P]
                 for o in range(2) for t in (t0, t0 + 1)],
                k_T[:, :, t0 * P:(t0 + 2) * P].rearrange(
                    "p o (t c) -> p o t c", c=P))
        q_T = ph0.tile([P, DO, NQ], F32, name="q_T")
        for t0 in range(0, QT, 2):
            transpose4(
                [q_sb[:, t, o * P:(o + 1) * P]
                 for o in range(2) for t in (t0, t0 + 1)],
                q_T[:, :, t0 * P:(t0 + 2) * P].rearrange(
                    "p o (t c) -> p o t c", c=P))

        # K/Q projections -> bf16 transposed layout, bias via K=1 matmul
        def project(wT, src, b_row, dst, N):
            for o in range(DO):
                for c0 in range(0, N, 512):
                    ps = pps.tile([P, 512], F32, tag=f"pj{(c0 // 512) % 2}",
                                  name="pj")
                    for ki in range(DO):
                        nc.tensor.matmul(
                            ps, lhsT=wT[:, ki, o * P:(o + 1) * P],
                            rhs=src[:, ki, c0:c0 + 512],
                            start=(ki == 0), stop=False)
                    nc.tensor.matmul(
                        ps, lhsT=b_row[0:1, o * P:(o + 1) * P],
                        rhs=ones_r[0:1, :], start=False, stop=True)
                    drain(dst[:, o, c0:c0 + 512], ps)

        project(wkT, k_T, bk_row, K_bf, NK)
        project(wqT, q_T, bq_row, Q_bf, NQ)

        # V projection: natural layout [tok, dv] per key tile
        for t in range(KT):
            ps = pps.tile([P, D], F32, tag=f"vp{t % 2}", name="vp")
            for ki in range(DO):
                nc.tensor.matmul(
                    ps, lhsT=k_T[:, ki, t * P:(t + 1) * P], rhs=wvT[:, ki, :],
                    start=(ki == 0), stop=False)
            nc.tensor.matmul(ps, lhsT=ones_r[0:1, :P], rhs=bv_row[0:1, :],
                             start=False, stop=True)
            drain(V_nat[:, t, :, :].rearrange("p h w -> p (h w)"), ps)

    # =================== attention ===================
    exp_split = EngineSplit(ACT_EXP_SHARE)
    with tc.tile_pool(name="attn_sb", bufs=1) as asb, \
         tc.tile_pool(name="sps", bufs=1, space="PSUM") as sps, \
         tc.tile_pool(name="ops", bufs=1, space="PSUM") as ops, \
         tc.tile_pool(name="nrm", bufs=2) as nrm:
        attn01 = asb.tile([P, KT, 2, SC], BF16, name="attn01")
        attn23 = asb.tile([P, KT, 2, SC], BF16, name="attn23")

        for qc in range(cfg.QCN):
            qsl = slice(qc * SC, (qc + 1) * SC)
            for o in range(DO):
                # ---- scores + exp ----
                for kt in range(KT):
                    ksl = slice(kt * P, (kt + 1) * P)
                    pj = (2 * kt) % 3
                    pa = sps.tile([P, 2, SC], F32, tag=f"sp{pj}", name="pa")
                    pb = sps.tile([P, 2, SC], F32, tag=f"sp{(pj + 1) % 3}",
                                  name="pb")
                    for m, (pt, sl) in enumerate(
                            ((pa, 0), (pa, 1), (pb, 0), (pb, 1))):
                        nc.tensor.matmul(
                            pt[:, sl, :],
                            lhsT=K_bf[32 * m:32 * m + 32, o, ksl],
                            rhs=Q_bf[32 * m:32 * m + 32, o, qsl],
                            start=True, stop=True,
                            tile_position=(32 * m, 0))
                    for at, pt in ((attn01, pa), (attn23, pb)):
                        if exp_split.take():
                            nc.scalar.activation(at[:, kt, :, :], pt, AF.Exp,
                                                 scale=1.0 / 16.0)
                        else:
                            nc.vector.tensor_scalar(
                                at[:, kt, :, :].bitcast(I16), pt,
                                EXP_A, EXP_B, OP.mult, OP.add)

                # ---- PV + denominator (interleaved per key tile) ----
                po = ops.tile([P, SC], F32, tag="po", name="po")
                pd = ops.tile([P, SC], F32, tag="pd", name="pd")
                for t in range(KT):
                    for m in range(4):
                        at = attn01 if m < 2 else attn23
                        rhs = at[:, t, m % 2, :]
                        nc.tensor.matmul(
                            po[32 * m:32 * m + 32, :],
                            lhsT=V_nat[:, t, o * 4 + m, :], rhs=rhs,
                            start=(t == 0), stop=(t == KT - 1),
                            tile_position=(0, 32 * m), skip_group_check=True)
                    for m in range(4):
                        at = attn01 if m < 2 else attn23
                        rhs = at[:, t, m % 2, :]
                        nc.tensor.matmul(
                            pd[32 * m:32 * m + 32, :], lhsT=ones32, rhs=rhs,
                            start=(t == 0), stop=(t == KT - 1),
                            tile_position=(0, 32 * m), skip_group_check=True)

                rec = nrm.tile([P, SC], F32, tag="rec", name="rec")
                nc.vector.reciprocal_approx_fast(rec, pd)
                osl = O_T[:, o, qsl]
                nc.vector.tensor_tensor(osl, po, rec, OP.mult)
                nc.vector.tensor_tensor(osl, osl, Q_bf[:, o, qsl], OP.add)

    # =================== epilogue: LN0, FFN, LN1, out ===================
    with tc.tile_pool(name="ep_sb", bufs=1) as esb, \
         tc.tile_pool(name="st_ps", bufs=1, space="PSUM") as stp, \
         tc.tile_pool(name="bc_ps", bufs=1, space="PSUM") as bcp, \
         tc.tile_pool(name="f_ps", bufs=1, space="PSUM") as fps, \
         tc.tile_pool(name="o_ps", bufs=1, space="PSUM") as otp:
        NC = LC // P  # 4 query-pieces per chunk

        def ln_rows(src, csl, tag):
            """LN stats for src[:, :, csl] -> (A8, B8) [128, NC] f32 tiles
            holding rstd and -mu*rstd per query (q = piece*128 + p)."""
            x2 = esb.tile([P, DO, LC], F32, tag=f"x2{tag}", name="x2")
            for o in range(DO):
                nc.scalar.activation(x2[:, o, :], src[:, o, csl], AF.Square)
            st_sb = esb.tile([1, 2, LC], F32, tag=f"stsb{tag}", name="st_sb")
            for i in range(2):
                st = stp.tile([1, LC], F32, tag="st", name="st")
                for o in range(DO):
                    rhs = src[:, o, csl] if i == 0 else x2[:, o, :]
                    nc.tensor.matmul(st, lhsT=ones_k, rhs=rhs,
                                     start=(o == 0), stop=(o == DO - 1))
                nc.scalar.copy(st_sb[:, i, :], st)
            # transpose pieces into [128, 2*NC] (sx | sx2)
            st_t = stp.tile([P, 2 * NC], F32, tag="stt", name="st_t")
            for i in range(2):
                for j in range(NC):
                    nc.tensor.matmul(
                        st_t[:, i * NC + j:i * NC + j + 1],
                        lhsT=st_sb[0:1, i, j * P:(j + 1) * P],
                        rhs=ones_r[0:1, 0:1], start=True, stop=True)
            stt_sb = esb.tile([P, 2 * NC], F32, tag=f"sttsb{tag}", name="stt_sb")
            nc.vector.tensor_copy(stt_sb, st_t)
            sx, sx2 = stt_sb[:, 0:NC], stt_sb[:, NC:2 * NC]
            r8 = esb.tile([P, 5, NC], F32, tag=f"r8{tag}", name="r8")
            mu, ve, var_e, y2t, A8 = (r8[:, i, :] for i in range(5))
            nc.vector.tensor_scalar(mu, sx, 1.0 / D, None, OP.mult)
            nc.vector.tensor_scalar(ve, sx2, 1.0 / D, EPS, OP.mult, OP.add)
            nc.vector.tensor_tensor(var_e, mu, mu, OP.mult)
            nc.vector.tensor_tensor(var_e, ve, var_e, OP.subtract)
            y0 = esb.tile([P, NC], I16, tag=f"y0{tag}", name="y0")
            nc.vector.tensor_scalar(y0, var_e[:].bitcast(I32),
                                    RS_A, RS_B, OP.mult, OP.add)
            y0b = y0[:].bitcast(BF16)  # ~= rstd seed (+-3.7%)
            nc.vector.tensor_tensor(y2t, y0b, y0b, OP.mult)
            nc.vector.tensor_tensor(y2t, y2t, var_e, OP.mult)
            nc.vector.tensor_scalar(y2t, y2t, -0.5, 1.5, OP.mult, OP.add)
            nc.vector.tensor_tensor(A8, y2t, y0b, OP.mult)  # rstd
            B8 = r8[:, 3, :]  # reuse y2t slot
            nc.vector.scalar_tensor_tensor(B8, mu, -1.0, A8, OP.mult, OP.mult)
            return A8, B8

        for c in range(cfg.LCN):
            csl = slice(c * LC, (c + 1) * LC)
            # ---- LN0 ----
            A8, B8 = ln_rows(O_T, csl, f"a{c % 2}")
            # broadcast A/B to [P, LC]: transpose each piece to a
            # partition-0 row, then K=1 ones-matmul outer product
            abr_ps = stp.tile([1, 2 * NC, P], F32, tag="abr", name="abr_ps")
            for i, r8t in enumerate((A8, B8)):
                for j in range(NC):
                    nc.tensor.matmul(
                        abr_ps[:, i * NC + j, :], lhsT=r8t[:, j:j + 1],
                        rhs=ident, start=True, stop=True)
            abr = esb.tile([1, 2 * NC, P], F32, tag=f"abrs{c % 2}", name="abr")
            nc.vector.tensor_copy(abr, abr_ps)
            pab = bcp.tile([P, 2, LC], F32, tag="pab", name="pab")
            for i in range(2):
                for j in range(NC):
                    nc.tensor.matmul(
                        pab[:, i, j * P:(j + 1) * P], lhsT=ones_r[0:1, 0:P],
                        rhs=abr[0:1, i * NC + j, :],
                        start=True, stop=True)
            for o in range(DO):
                xsl = X0[:, o, csl]
                nc.vector.tensor_tensor(xsl, O_T[:, o, csl], pab[:, 0, :],
                                        OP.mult)
                nc.vector.tensor_tensor(xsl, xsl, pab[:, 1, :], OP.add)
                if cfg.ln0_gb:
                    nc.vector.scalar_tensor_tensor(
                        xsl, xsl, g0_pm[:, o:o + 1],
                        b0_pm[:, o:o + 1].to_broadcast([P, LC]),
                        OP.mult, OP.add)
            # ---- FFN ----
            for o in range(DO):
                fp = fps.tile([P, LC], F32, tag="f", name="fp")
                for ki in range(DO):
                    nc.tensor.matmul(fp, lhsT=woT[:, ki, o * P:(o + 1) * P],
                                     rhs=X0[:, ki, csl],
                                     start=(ki == 0), stop=(ki == DO - 1))
                h = esb.tile([P, LC], F32, tag=f"h{o}", name="h")
                nc.scalar.activation(h, fp, AF.Relu, bias=bo_pm[:, o:o + 1])
                nc.vector.tensor_tensor(X1[:, o, csl], X0[:, o, csl], h, OP.add)
            # ---- LN1 + transpose out ----
            A81, B81 = ln_rows(X1, csl, f"b{c % 2}")
            for j in range(NC):
                t = c * NC + j
                tp = otp.tile([P, 2, P], F32, tag="ot", name="tp")
                for o in range(DO):
                    nc.tensor.transpose(tp[:, o, :], X1[:, o, t * P:(t + 1) * P],
                                        ident)
                ov = out_nat[:, t, :].rearrange("p (o c2) -> p o c2", c2=P)
                nc.vector.tensor_scalar(ov, tp, A81[:, j:j + 1], B81[:, j:j + 1],
                                        OP.mult, OP.add)
                if cfg.ln1_gb:
                    ovf = out_nat[:, t, :]
                    nc.vector.tensor_tensor(ovf, ovf, g1_bc, OP.mult)
                    nc.vector.tensor_tensor(ovf, ovf, b1_bc, OP.add)
        nc.sync.dma_start(io["out"][:].rearrange("(t p) d -> p t d", p=P),
                          out_nat)


def build(cfg: Cfg) -> bass.Bass:
    nc = bacc.Bacc("TRN2")
    io = {}
    for name, shape in (
        ("q", [cfg.NQ, cfg.D]), ("k", [cfg.NK, cfg.D]),
        ("Wq", [cfg.D, cfg.D]), ("Wk", [cfg.D, cfg.D]),
        ("Wv", [cfg.D, cfg.D]), ("Wo", [cfg.D, cfg.D]),
        ("bq", [cfg.D]), ("bk", [cfg.D]), ("bv", [cfg.D]), ("bo", [cfg.D]),
        ("g0", [cfg.D]), ("b0", [cfg.D]), ("g1", [cfg.D]), ("b1", [cfg.D]),
    ):
        io[name] = nc.dram_tensor(name, shape, F32, kind="ExternalInput")
    io["out"] = nc.dram_tensor("out", [cfg.NQ, cfg.D], F32, kind="ExternalOutput")

    with tile.TileContext(nc) as tc:
        with ExitStack() as ctx:
            _emit(nc, tc, ctx, io, cfg)
    nc.compile()
    return nc


_CACHE = {}


def _get_nc(key, cfg):
    if key not in _CACHE:
        _CACHE[key] = build(cfg)
    return _CACHE[key]


def kernel(q, k, Wq, bq, Wk, bk, Wv, bv, Wo, bo, g0, b0, g1, b1, _trace=False):
    from concourse.bass_utils import run_bass_kernel_spmd

    B, Nq, D = q.shape
    Nk = k.shape[1]
    n_cores = 8
    halves = n_cores // B
    nq_c = Nq // halves
    ln0_gb = not (np.allclose(g0, 1.0) and np.allclose(b0, 0.0))
    ln1_gb = not (np.allclose(g1, 1.0) and np.allclose(b1, 0.0))
    cfg = Cfg(NQ=nq_c, NK=Nk, D=D, ln0_gb=ln0_gb, ln1_gb=ln1_gb)
    nc = _get_nc((nq_c, Nk, D, ln0_gb, ln1_gb), cfg)

    shared = dict(Wq=Wq, bq=bq, Wk=Wk, bk=bk, Wv=Wv, bv=bv, Wo=Wo, bo=bo,
                  g0=g0, b0=b0, g1=g1, b1=b1)
    shared = {n: np.ascontiguousarray(v, dtype=np.float32)
              for n, v in shared.items()}
    in_maps = []
    for c in range(n_cores):
        b, hf = c // halves, c % halves
        m = dict(shared)
        m["q"] = np.ascontiguousarray(q[b, hf * nq_c:(hf + 1) * nq_c], np.float32)
        m["k"] = np.ascontiguousarray(k[b], np.float32)
        in_maps.append(m)

    res = run_bass_kernel_spmd(nc, in_maps, core_ids=list(range(n_cores)),
                               trace=_trace)
    out = np.empty((B, Nq, D), np.float32)
    for c in range(n_cores):
        b, hf = c // halves, c % halves
        out[b, hf * nq_c:(hf + 1) * nq_c] = res.results[c]["out"]
    if _trace:
        return out, res
    return out


# revision 14
# speedup vs baseline: 2.0561x; 1.3980x over previous
"""MAB (Set-Transformer multihead attention block) Trainium2 Bass kernel, v3.

Reference math (fp32):
  Q = q @ Wq.T + bq ; K = k @ Wk.T + bk ; V = k @ Wv.T + bv    [B,N,256]
  per head h (8 heads x 32): s = Qh @ Kh.T / 16 ; a = softmax(s)
  Oh = Qh + a @ Vh ; o = concat(Oh) ; o = LN0(o) ; o = o + relu(o @ Wo.T + bo)
  out = LN1(o)

Sharding: 8 cores = (batch b in 0..3, query-half in 0..1); no collectives.

v3 vs v2:
  - everything bf16 on the PE (fp32 matmuls are 2-pass LOW_HIGH = half
    rate): inputs cast to bf16 right after DMA, transposes/projections/
    stats/FFN/output-transposes all bf16 operands, fp32 PSUM accumulate.
  - PV + denominator matmuls interleaved into the score loop at lag 1,
    so the PE fills the gaps while ACT/DVE pace the exp drains (the
    attention wall is the exp-drain throughput).
  - scores: 4-way row-tiled concurrent K=32 matmuls per head o-block.
  - exp split ACT (table exp) / DVE (bf16 bit-trick, f32->i16 rounding).
  - biases folded into K=1 ones-row matmuls; drains are plain copies.
  - LN row math in a [128, pieces] transposed layout; rstd via a direct
    rsqrt bit-trick + 1 Newton step (no Ln/Sqrt tables; single
    ACT_TABLE_LOAD in the whole kernel).
  - LN1 apply fused into the output transpose drains.
"""

import os
import sys
from contextlib import ExitStack

import numpy as np

for _p in ("/opt/trn_rl_repo", "/root/.axon_site/_ro/trn_rl_repo"):
    if os.path.isdir(_p) and _p not in sys.path:
        sys.path.insert(0, _p)

import concourse.bass as bass  # noqa: E402
import concourse.tile as tile  # noqa: E402
from concourse import bacc, mybir  # noqa: E402
from concourse.masks import make_identity  # noqa: E402

F32 = mybir.dt.float32
BF16 = mybir.dt.bfloat16
I16 = mybir.dt.int16
I32 = mybir.dt.int32
P = 128
EPS = 1e-5

AF = mybir.ActivationFunctionType
OP = mybir.AluOpType

LOG2E = 1.4426950408889634
# exp(x/16) ~= bf16(bits = round(x*EXP_A + EXP_B))
EXP_A = 128.0 * LOG2E / 16.0
EXP_B = 128.0 * (127.0 - 0.04380)
# rsqrt(v) ~= bf16(bits = round(bits32(v)*RS_A + RS_B)), then 1 Newton step
RS_A = -64.0 / (1 << 23)
RS_B = 24375.283445

ACT_EXP_SHARE = 0.56  # fraction of exp drains on ScalarE


class Cfg:
    def __init__(self, NQ=1024, NK=2048, D=256, H=8, ln0_gb=False, ln1_gb=False):
        self.NQ, self.NK, self.D, self.H = NQ, NK, D, H
        self.HD = D // H            # 32
        self.DO = D // P            # 2
        self.QT = NQ // P           # 8
        self.KT = NK // P           # 16
        self.SC = 512               # score/attn q-chunk
        self.QCN = NQ // self.SC    # 2
        self.LC = 512               # ln/ffn q-chunk
        self.LCN = NQ // self.LC
        self.ln0_gb = ln0_gb        # apply non-trivial g0/b0
        self.ln1_gb = ln1_gb
        assert self.HD == 32 and self.DO == 2


class EngineSplit:
    """Bresenham-style assigner: returns True for ACT with given share."""

    def __init__(self, share):
        self.share = share
        self.acc = 0.0

    def take(self):
        self.acc += self.share
        if self.acc >= 1.0:
            self.acc -= 1.0
            return True
        return False


def _emit(nc: bass.Bass, tc: tile.TileContext, ctx: ExitStack, io: dict, cfg: Cfg):
    NQ, NK, D, H = cfg.NQ, cfg.NK, cfg.D, cfg.H
    DO, QT, KT, SC, LC = cfg.DO, cfg.QT, cfg.KT, cfg.SC, cfg.LC

    const = ctx.enter_context(tc.tile_pool(name="const", bufs=1))
    persist = ctx.enter_context(tc.tile_pool(name="persist", bufs=1))

    # ---- constants ----
    ident = const.tile([P, P], F32)
    make_identity(nc, ident)
    ident_bf = const.tile([P, P], BF16)
    nc.vector.tensor_copy(ident_bf, ident)
    ones32 = const.tile([P, 32], BF16)
    nc.vector.memset(ones32, 1.0)
    ones_r = const.tile([1, 512], BF16)  # ones row: bias-matmul rhs / lhsT
    nc.vector.memset(ones_r, 1.0)
    ones_rf = const.tile([1, P], F32)    # f32 ones row (broadcast matmuls)
    nc.vector.memset(ones_rf, 1.0)
    ones_k = const.tile([P, 1], BF16)    # LN stats lhsT (partition sum)
    nc.vector.memset(ones_k, 1.0)

    def vec_row(name, dtype=BF16):
        tf = const.tile([1, D], F32, name=f"{name}_rowf")
        nc.sync.dma_start(tf, io[name][:].rearrange("(o d) -> o d", o=1))
        if dtype == F32:
            return tf
        t = const.tile([1, D], BF16, name=f"{name}_row")
        nc.vector.tensor_copy(t, tf)
        return t

    bq_row, bk_row, bv_row = vec_row("bq"), vec_row("bk"), vec_row("bv")

    def vec_pm(name):
        t = const.tile([P, DO], F32, name=f"{name}_pm")
        nc.sync.dma_start(t, io[name][:].rearrange("(o p) -> p o", p=P))
        return t

    bo_pm = vec_pm("bo")
    g0_pm = vec_pm("g0") if cfg.ln0_gb else None
    b0_pm = vec_pm("b0") if cfg.ln0_gb else None
    if cfg.ln1_gb:
        g1_bc = const.tile([P, D], F32)
        nc.gpsimd.partition_broadcast(g1_bc, vec_row("g1", F32))
        b1_bc = const.tile([P, D], F32)
        nc.gpsimd.partition_broadcast(b1_bc, vec_row("b1", F32))

    # ---- persistent tensors (all bf16) ----
    Q_bf = persist.tile([P, DO, NQ], BF16, name="Q_bf")
    K_bf = persist.tile([P, DO, NK], BF16, name="K_bf")
    V_nat = persist.tile([P, KT, H, 32], BF16, name="V_nat")  # [tok, dv]/tile
    woT = persist.tile([P, DO, D], BF16, name="woT")
    O_bf = persist.tile([P, DO, NQ], BF16, name="O_bf")
    X0 = persist.tile([P, DO, NQ], BF16, name="X0")
    X1 = persist.tile([P, DO, NQ], BF16, name="X1")
    out_nat = persist.tile([P, QT, D], F32, name="out_nat")

    drain_split = EngineSplit(0.5)

    def drain(dst, src):
        if drain_split.take():
            nc.scalar.copy(dst, src)
        else:
            nc.vector.tensor_copy(dst, src)

    # =================== prologue: transposes + projections ===================
    with tc.tile_pool(name="ph0", bufs=1) as ph0, \
         tc.tile_pool(name="tps", bufs=1, space="PSUM") as tps, \
         tc.tile_pool(name="pps", bufs=1, space="PSUM") as pps:

        # DMA + cast to bf16 (W first: unblocks transposes earliest)
        def load_bf(name, shape, rearr, tag=None):
            tf = ph0.tile(shape, F32, name=f"{name}_f", tag=tag or f"{name}_f")
            nc.sync.dma_start(tf, io[name][:].rearrange(rearr, p=P))
            tb = ph0.tile(shape, BF16, name=f"{name}_b")
            nc.vector.tensor_copy(tb, tf)
            return tb

        w_bf = {n: load_bf(n, [P, DO, D], "(o p) f -> p o f", tag="wl")
                for n in ("Wk", "Wv", "Wq", "Wo")}
        k_bf = load_bf("k", [P, KT, D], "(t p) d -> p t d")
        q_bf = load_bf("q", [P, QT, D], "(t p) d -> p t d")

        tp_rot = [0]

        def transpose4(pieces, dst):
            """PE-transpose four bf16 [128,128] pieces into one psum bank,
            one drain into dst (a [P, 2, 2, 128]-shaped bf16 AP)."""
            ps = tps.tile([P, 4, P], BF16, tag=f"tp{tp_rot[0] % 3}", name="tp")
            tp_rot[0] += 1
            for j, src in enumerate(pieces):
                nc.tensor.transpose(ps[:, j, :], src, ident_bf)
            drain(dst, ps[:].rearrange("p (a b) c -> p a b c", a=2))

        def load_wT(name, dst):
            w_sb = w_bf[name]
            transpose4(
                [w_sb[:, o, fo * P:(fo + 1) * P]
                 for fo in range(2) for o in range(2)],
                dst[:].rearrange("p f (o c) -> p f o c", c=P))
            return dst

        wkT = load_wT("Wk", ph0.tile([P, DO, D], BF16, name="wkT"))
        wvT = load_wT("Wv", ph0.tile([P, DO, D], BF16, name="wvT"))
        wqT = load_wT("Wq", ph0.tile([P, DO, D], BF16, name="wqT"))
        load_wT("Wo", woT)

        k_T = ph0.tile([P, DO, NK], BF16, name="k_T")
        for t0 in range(0, KT, 2):
            transpose4(
                [k_bf[:, t, o * P:(o + 1) * P]
                 for o in range(2) for t in (t0, t0 + 1)],
                k_T[:, :, t0 * P:(t0 + 2) * P].rearrange(
                    "p o (t c) -> p o t c", c=P))
        q_T = ph0.tile([P, DO, NQ], BF16, name="q_T")
        for t0 in range(0, QT, 2):
            transpose4(
                [q_bf[:, t, o * P:(o + 1) * P]
                 for o in range(2) for t in (t0, t0 + 1)],
                q_T[:, :, t0 * P:(t0 + 2) * P].rearrange(
                    "p o (t c) -> p o t c", c=P))

        # K/Q projections -> bf16 transposed layout, bias via K=1 matmul
        def project(wT, src, b_row, dst, N):
            for o in range(DO):
                for c0 in range(0, N, 512):
                    ps = pps.tile([P, 512], F32, tag=f"pj{(c0 // 512) % 2}",
                                  name="pj")
                    for ki in range(DO):
                        nc.tensor.matmul(
                            ps, lhsT=wT[:, ki, o * P:(o + 1) * P],
                            rhs=src[:, ki, c0:c0 + 512],
                            start=(ki == 0), stop=False)
                    nc.tensor.matmul(
                        ps, lhsT=b_row[0:1, o * P:(o + 1) * P],
                        rhs=ones_r[0:1, :], start=False, stop=True)
                    drain(dst[:, o, c0:c0 + 512], ps)

        project(wkT, k_T, bk_row, K_bf, NK)
        project(wqT, q_T, bq_row, Q_bf, NQ)

        # V projection: natural layout [tok, dv] per key tile
        for t in range(KT):
            ps = pps.tile([P, D], F32, tag=f"vp{t % 2}", name="vp")
            for ki in range(DO):
                nc.tensor.matmul(
                    ps, lhsT=k_T[:, ki, t * P:(t + 1) * P], rhs=wvT[:, ki, :],
                    start=(ki == 0), stop=False)
            nc.tensor.matmul(ps, lhsT=ones_r[0:1, :P], rhs=bv_row[0:1, :],
                             start=False, stop=True)
            drain(V_nat[:, t, :, :].rearrange("p h w -> p (h w)"), ps)

    # =================== attention ===================
    exp_split = EngineSplit(ACT_EXP_SHARE)
    with tc.tile_pool(name="attn_sb", bufs=1) as asb, \
         tc.tile_pool(name="sps", bufs=1, space="PSUM") as sps, \
         tc.tile_pool(name="ops", bufs=1, space="PSUM") as ops, \
         tc.tile_pool(name="nrm", bufs=2) as nrm:
        attn01 = asb.tile([P, KT, 2, SC], BF16, name="attn01")
        attn23 = asb.tile([P, KT, 2, SC], BF16, name="attn23")

        for qc in range(cfg.QCN):
            qsl = slice(qc * SC, (qc + 1) * SC)
            for o in range(DO):
                po = ops.tile([P, SC], F32, tag="po", name="po")
                pd = ops.tile([P, SC], F32, tag="pd", name="pd")

                def pv_step(t, po=po, pd=pd, o=o):
                    for m in range(4):
                        at = attn01 if m < 2 else attn23
                        rhs = at[:, t, m % 2, :]
                        nc.tensor.matmul(
                            po[32 * m:32 * m + 32, :],
                            lhsT=V_nat[:, t, o * 4 + m, :], rhs=rhs,
                            start=(t == 0), stop=(t == KT - 1),
                            tile_position=(0, 32 * m), skip_group_check=True)
                    for m in range(4):
                        at = attn01 if m < 2 else attn23
                        rhs = at[:, t, m % 2, :]
                        nc.tensor.matmul(
                            pd[32 * m:32 * m + 32, :], lhsT=ones32, rhs=rhs,
                            start=(t == 0), stop=(t == KT - 1),
                            tile_position=(0, 32 * m), skip_group_check=True)

                # scores + exp, with PV interleaved at lag 1
                for kt in range(KT):
                    ksl = slice(kt * P, (kt + 1) * P)
                    pj = (2 * kt) % 3
                    pa = sps.tile([P, 2, SC], F32, tag=f"sp{pj}", name="pa")
                    pb = sps.tile([P, 2, SC], F32, tag=f"sp{(pj + 1) % 3}",
                                  name="pb")
                    for m, (pt, sl) in enumerate(
                            ((pa, 0), (pa, 1), (pb, 0), (pb, 1))):
                        nc.tensor.matmul(
                            pt[:, sl, :],
                            lhsT=K_bf[32 * m:32 * m + 32, o, ksl],
                            rhs=Q_bf[32 * m:32 * m + 32, o, qsl],
                            start=True, stop=True,
                            tile_position=(32 * m, 0))
                    for at, pt in ((attn01, pa), (attn23, pb)):
                        if exp_split.take():
                            nc.scalar.activation(at[:, kt, :, :], pt, AF.Exp,
                                                 scale=1.0 / 16.0)
                        else:
                            nc.vector.tensor_scalar(
                                at[:, kt, :, :].bitcast(I16), pt,
                                EXP_A, EXP_B, OP.mult, OP.add)
                    if kt >= 1:
                        pv_step(kt - 1)
                pv_step(KT - 1)

                rec = nrm.tile([P, SC], F32, tag="rec", name="rec")
                nc.vector.reciprocal_approx_fast(rec, pd)
                osl = O_bf[:, o, qsl]
                nc.vector.tensor_tensor(osl, po, rec, OP.mult)
                nc.vector.tensor_tensor(osl, osl, Q_bf[:, o, qsl], OP.add)

    # =================== epilogue: LN0, FFN, LN1, out ===================
    with tc.tile_pool(name="ep_sb", bufs=1) as esb, \
         tc.tile_pool(name="st_ps", bufs=1, space="PSUM") as stp, \
         tc.tile_pool(name="bc_ps", bufs=1, space="PSUM") as bcp, \
         tc.tile_pool(name="f_ps", bufs=1, space="PSUM") as fps, \
         tc.tile_pool(name="o_ps", bufs=1, space="PSUM") as otp:
        NC = LC // P  # 4 query-pieces per chunk

        def ln_rows(src, csl, tag):
            """LN stats for src[:, :, csl] -> (A8, B8) [128, NC] f32 tiles
            holding rstd and -mu*rstd per query (q = piece*128 + p)."""
            x2 = esb.tile([P, DO, LC], BF16, tag=f"x2{tag}", name="x2")
            for o in range(DO):
                nc.scalar.activation(x2[:, o, :], src[:, o, csl], AF.Square)
            st_sb = esb.tile([1, 2, LC], F32, tag=f"stsb{tag}", name="st_sb")
            for i in range(2):
                st = stp.tile([1, LC], F32, tag="st", name="st")
                for o in range(DO):
                    rhs = src[:, o, csl] if i == 0 else x2[:, o, :]
                    nc.tensor.matmul(st, lhsT=ones_k, rhs=rhs,
                                     start=(o == 0), stop=(o == DO - 1))
                nc.scalar.copy(st_sb[:, i, :], st)
            # transpose pieces into [128, 2*NC] (sx | sx2)
            st_t = stp.tile([P, 2 * NC], F32, tag="stt", name="st_t")
            for i in range(2):
                for j in range(NC):
                    nc.tensor.matmul(
                        st_t[:, i * NC + j:i * NC + j + 1],
                        lhsT=st_sb[0:1, i, j * P:(j + 1) * P],
                        rhs=ones_rf[0:1, 0:1], start=True, stop=True)
            stt_sb = esb.tile([P, 2 * NC], F32, tag=f"sttsb{tag}", name="stt_sb")
            nc.vector.tensor_copy(stt_sb, st_t)
            sx, sx2 = stt_sb[:, 0:NC], stt_sb[:, NC:2 * NC]
            r8 = esb.tile([P, 5, NC], F32, tag=f"r8{tag}", name="r8")
            mu, ve, var_e, y2t, A8 = (r8[:, i, :] for i in range(5))
            nc.vector.tensor_scalar(mu, sx, 1.0 / D, None, OP.mult)
            nc.vector.tensor_scalar(ve, sx2, 1.0 / D, EPS, OP.mult, OP.add)
            nc.vector.tensor_tensor(var_e, mu, mu, OP.mult)
            nc.vector.tensor_tensor(var_e, ve, var_e, OP.subtract)
            y0 = esb.tile([P, NC], I16, tag=f"y0{tag}", name="y0")
            nc.vector.tensor_scalar(y0, var_e[:].bitcast(I32),
                                    RS_A, RS_B, OP.mult, OP.add)
            y0b = y0[:].bitcast(BF16)  # ~= rstd seed (+-3.7%)
            nc.vector.tensor_tensor(y2t, y0b, y0b, OP.mult)
            nc.vector.tensor_tensor(y2t, y2t, var_e, OP.mult)
            nc.vector.tensor_scalar(y2t, y2t, -0.5, 1.5, OP.mult, OP.add)
            nc.vector.tensor_tensor(A8, y2t, y0b, OP.mult)  # rstd
            B8 = r8[:, 3, :]  # reuse y2t slot
            nc.vector.scalar_tensor_tensor(B8, mu, -1.0, A8, OP.mult, OP.mult)
            return A8, B8

        for c in range(cfg.LCN):
            csl = slice(c * LC, (c + 1) * LC)
            # ---- LN0 ----
            A8, B8 = ln_rows(O_bf, csl, f"a{c % 2}")
            # broadcast A/B to [P, LC]: transpose each piece to a
            # partition-0 row, then K=1 ones-matmul outer product
            abr_ps = stp.tile([1, 2 * NC, P], F32, tag="abr", name="abr_ps")
            for i, r8t in enumerate((A8, B8)):
                for j in range(NC):
                    nc.tensor.matmul(
                        abr_ps[:, i * NC + j, :], lhsT=r8t[:, j:j + 1],
                        rhs=ident, start=True, stop=True)
            abr = esb.tile([1, 2 * NC, P], F32, tag=f"abrs{c % 2}", name="abr")
            nc.vector.tensor_copy(abr, abr_ps)
            pab = bcp.tile([P, 2, LC], F32, tag="pab", name="pab")
            for i in range(2):
                for j in range(NC):
                    nc.tensor.matmul(
                        pab[:, i, j * P:(j + 1) * P], lhsT=ones_rf[0:1, 0:P],
                        rhs=abr[0:1, i * NC + j, :],
                        start=True, stop=True)
            for o in range(DO):
                xsl = X0[:, o, csl]
                nc.vector.tensor_tensor(xsl, O_bf[:, o, csl], pab[:, 0, :],
                                        OP.mult)
                nc.vector.tensor_tensor(xsl, xsl, pab[:, 1, :], OP.add)
                if cfg.ln0_gb:
                    nc.vector.scalar_tensor_tensor(
                        xsl, xsl, g0_pm[:, o:o + 1],
                        b0_pm[:, o:o + 1].to_broadcast([P, LC]),
                        OP.mult, OP.add)
            # ---- FFN ----
            for o in range(DO):
                fp = fps.tile([P, LC], F32, tag="f", name="fp")
                for ki in range(DO):
                    nc.tensor.matmul(fp, lhsT=woT[:, ki, o * P:(o + 1) * P],
                                     rhs=X0[:, ki, csl],
                                     start=(ki == 0), stop=(ki == DO - 1))
                h = esb.tile([P, LC], BF16, tag=f"h{o}", name="h")
                nc.scalar.activation(h, fp, AF.Relu, bias=bo_pm[:, o:o + 1])
                nc.vector.tensor_tensor(X1[:, o, csl], X0[:, o, csl], h, OP.add)
            # ---- LN1 + transpose out ----
            A81, B81 = ln_rows(X1, csl, f"b{c % 2}")
            for j in range(NC):
                t = c * NC + j
                tp = otp.tile([P, 2, P], BF16, tag="ot", name="tp")
                for o in range(DO):
                    nc.tensor.transpose(tp[:, o, :],
                                        X1[:, o, t * P:(t + 1) * P], ident_bf)
                ov = out_nat[:, t, :].rearrange("p (o c2) -> p o c2", c2=P)
                nc.vector.tensor_scalar(ov, tp, A81[:, j:j + 1], B81[:, j:j + 1],
                                        OP.mult, OP.add)
                if cfg.ln1_gb:
                    ovf = out_nat[:, t, :]
                    nc.vector.tensor_tensor(ovf, ovf, g1_bc, OP.mult)
                    nc.vector.tensor_tensor(ovf, ovf, b1_bc, OP.add)
        nc.sync.dma_start(io["out"][:].rearrange("(t p) d -> p t d", p=P),
                          out_nat)


def build(cfg: Cfg) -> bass.Bass:
    nc = bacc.Bacc("TRN2")
    io = {}
    for name, shape in (
        ("q", [cfg.NQ, cfg.D]), ("k", [cfg.NK, cfg.D]),
        ("Wq", [cfg.D, cfg.D]), ("Wk", [cfg.D, cfg.D]),
        ("Wv", [cfg.D, cfg.D]), ("Wo", [cfg.D, cfg.D]),
        ("bq", [cfg.D]), ("bk", [cfg.D]), ("bv", [cfg.D]), ("bo", [cfg.D]),
        ("g0", [cfg.D]), ("b0", [cfg.D]), ("g1", [cfg.D]), ("b1", [cfg.D]),
    ):
        io[name] = nc.dram_tensor(name, shape, F32, kind="ExternalInput")
    io["out"] = nc.dram_tensor("out", [cfg.NQ, cfg.D], F32, kind="ExternalOutput")

    with tile.TileContext(nc) as tc:
        with ExitStack() as ctx:
            _emit(nc, tc, ctx, io, cfg)
    nc.compile()
    return nc


_CACHE = {}


def _get_nc(key, cfg):
    if key not in _CACHE:
        _CACHE[key] = build(cfg)
    return _CACHE[key]


def kernel(q, k, Wq, bq, Wk, bk, Wv, bv, Wo, bo, g0, b0, g1, b1, _trace=False):
    from concourse.bass_utils import run_bass_kernel_spmd

    B, Nq, D = q.shape
    Nk = k.shape[1]
    n_cores = 8
    halves = n_cores // B
    nq_c = Nq // halves
    ln0_gb = not (np.allclose(g0, 1.0) and np.allclose(b0, 0.0))
    ln1_gb = not (np.allclose(g1, 1.0) and np.allclose(b1, 0.0))
    cfg = Cfg(NQ=nq_c, NK=Nk, D=D, ln0_gb=ln0_gb, ln1_gb=ln1_gb)
    nc = _get_nc((nq_c, Nk, D, ln0_gb, ln1_gb), cfg)

    shared = dict(Wq=Wq, bq=bq, Wk=Wk, bk=bk, Wv=Wv, bv=bv, Wo=Wo, bo=bo,
                  g0=g0, b0=b0, g1=g1, b1=b1)
    shared = {n: np.ascontiguousarray(v, dtype=np.float32)
              for n, v in shared.items()}
    in_maps = []
    for c in range(n_cores):
        b, hf = c // halves, c % halves
        m = dict(shared)
        m["q"] = np.ascontiguousarray(q[b, hf * nq_c:(hf + 1) * nq_c], np.float32)
        m["k"] = np.ascontiguousarray(k[b], np.float32)
        in_maps.append(m)

    res = run_bass_kernel_spmd(nc, in_maps, core_ids=list(range(n_cores)),
                               trace=_trace)
    out = np.empty((B, Nq, D), np.float32)
    for c in range(n_cores):
        b, hf = c // halves, c % halves
        out[b, hf * nq_c:(hf + 1) * nq_c] = res.results[c]["out"]
    if _trace:
        return out, res
    return out


# revision 16
# speedup vs baseline: 2.1871x; 1.0637x over previous
"""MAB (Set-Transformer multihead attention block) Trainium2 Bass kernel, v4.

Reference math (fp32):
  Q = q @ Wq.T + bq ; K = k @ Wk.T + bk ; V = k @ Wv.T + bv    [B,N,256]
  per head h (8 heads x 32): s = Qh @ Kh.T / 16 ; a = softmax(s)
  Oh = Qh + a @ Vh ; o = concat(Oh) ; o = LN0(o) ; o = o + relu(o @ Wo.T + bo)
  out = LN1(o)

Sharding: 8 cores = (batch b in 0..3, query-half in 0..1); no collectives.

v4 vs v3:
  - q/k/W are pre-transposed and pre-cast to bf16 on the host (input
    layout prep in the kernel() wrapper): no on-device input transposes
    or casts, and half the DMA bytes. Device inputs are qT/kT/W*T bf16.
  - epilogue rewritten: every matmul operand bf16 (no more fp32
    LOW_HIGH two-pass matmuls anywhere), LN0 broadcast tensors drained
    to SBUF bf16 so the apply runs at DVE 2x, and the two LN/FFN chunks
    are emitted stage-interleaved so their serial chains overlap.
  - attention unchanged from v3: 4-way row-tiled concurrent score
    matmuls, PV/denominator col-tiled matmuls interleaved at lag 1,
    exp split ACT (table exp) / DVE (bf16 bit-trick), denominators via
    ones matmuls, 1/d via reciprocal_approx_fast, rstd via rsqrt
    bit-trick + Newton (single ACT table load in the whole kernel).
"""

import os
import sys
from contextlib import ExitStack

import numpy as np

for _p in ("/opt/trn_rl_repo", "/root/.axon_site/_ro/trn_rl_repo"):
    if os.path.isdir(_p) and _p not in sys.path:
        sys.path.insert(0, _p)

import ml_dtypes  # noqa: E402
import concourse.bass as bass  # noqa: E402
import concourse.tile as tile  # noqa: E402
from concourse import bacc, mybir  # noqa: E402
from concourse.masks import make_identity  # noqa: E402

F32 = mybir.dt.float32
BF16 = mybir.dt.bfloat16
I16 = mybir.dt.int16
I32 = mybir.dt.int32
P = 128
EPS = 1e-5

AF = mybir.ActivationFunctionType
OP = mybir.AluOpType

LOG2E = 1.4426950408889634
# exp(x/16) ~= bf16(bits = round(x*EXP_A + EXP_B))
EXP_A = 128.0 * LOG2E / 16.0
EXP_B = 128.0 * (127.0 - 0.04380)
# rsqrt(v) ~= bf16(bits = round(bits32(v)*RS_A + RS_B)), then 1 Newton step
RS_A = -64.0 / (1 << 23)
RS_B = 24375.283445

ACT_EXP_SHARE = 0.56  # fraction of exp drains on ScalarE


class Cfg:
    def __init__(self, NQ=1024, NK=2048, D=256, H=8, ln0_gb=False, ln1_gb=False):
        self.NQ, self.NK, self.D, self.H = NQ, NK, D, H
        self.HD = D // H            # 32
        self.DO = D // P            # 2
        self.QT = NQ // P           # 8
        self.KT = NK // P           # 16
        self.SC = 512               # score/attn q-chunk
        self.QCN = NQ // self.SC    # 2
        self.LC = 512               # ln/ffn q-chunk
        self.LCN = NQ // self.LC
        self.ln0_gb = ln0_gb        # apply non-trivial g0/b0
        self.ln1_gb = ln1_gb
        assert self.HD == 32 and self.DO == 2


class EngineSplit:
    """Bresenham-style assigner: returns True for ACT with given share."""

    def __init__(self, share):
        self.share = share
        self.acc = 0.0

    def take(self):
        self.acc += self.share
        if self.acc >= 1.0:
            self.acc -= 1.0
            return True
        return False


def _emit(nc: bass.Bass, tc: tile.TileContext, ctx: ExitStack, io: dict, cfg: Cfg):
    NQ, NK, D, H = cfg.NQ, cfg.NK, cfg.D, cfg.H
    DO, QT, KT, SC, LC = cfg.DO, cfg.QT, cfg.KT, cfg.SC, cfg.LC

    const = ctx.enter_context(tc.tile_pool(name="const", bufs=1))
    persist = ctx.enter_context(tc.tile_pool(name="persist", bufs=1))

    # ---- constants ----
    ident = const.tile([P, P], F32)
    make_identity(nc, ident)
    ident_bf = const.tile([P, P], BF16)
    nc.vector.tensor_copy(ident_bf, ident)
    ones32 = const.tile([P, 32], BF16)
    nc.vector.memset(ones32, 1.0)
    ones_r = const.tile([1, 512], BF16)  # ones row: bias-matmul rhs / lhsT
    nc.vector.memset(ones_r, 1.0)
    ones_k = const.tile([P, 1], BF16)    # LN stats lhsT (partition sum)
    nc.vector.memset(ones_k, 1.0)

    def vec_row(name, dtype=BF16):
        tf = const.tile([1, D], F32, name=f"{name}_rowf")
        nc.sync.dma_start(tf, io[name][:].rearrange("(o d) -> o d", o=1))
        if dtype == F32:
            return tf
        t = const.tile([1, D], BF16, name=f"{name}_row")
        nc.vector.tensor_copy(t, tf)
        return t

    bq_row, bk_row, bv_row = vec_row("bq"), vec_row("bk"), vec_row("bv")

    def vec_pm(name):
        t = const.tile([P, DO], F32, name=f"{name}_pm")
        nc.sync.dma_start(t, io[name][:].rearrange("(o p) -> p o", p=P))
        return t

    bo_pm = vec_pm("bo")
    g0_pm = vec_pm("g0") if cfg.ln0_gb else None
    b0_pm = vec_pm("b0") if cfg.ln0_gb else None
    if cfg.ln1_gb:
        g1_bc = const.tile([P, D], F32)
        nc.gpsimd.partition_broadcast(g1_bc, vec_row("g1", F32))
        b1_bc = const.tile([P, D], F32)
        nc.gpsimd.partition_broadcast(b1_bc, vec_row("b1", F32))

    # ---- persistent tensors (all bf16) ----
    Q_bf = persist.tile([P, DO, NQ], BF16, name="Q_bf")
    K_bf = persist.tile([P, DO, NK], BF16, name="K_bf")
    V_nat = persist.tile([P, KT, H, 32], BF16, name="V_nat")  # [tok, dv]/tile
    woT = persist.tile([P, DO, D], BF16, name="woT")
    O_bf = persist.tile([P, DO, NQ], BF16, name="O_bf")
    X0 = persist.tile([P, DO, NQ], BF16, name="X0")
    X1 = persist.tile([P, DO, NQ], BF16, name="X1")
    out_nat = persist.tile([P, QT, D], F32, name="out_nat")

    drain_split = EngineSplit(0.5)

    def drain(dst, src):
        if drain_split.take():
            nc.scalar.copy(dst, src)
        else:
            nc.vector.tensor_copy(dst, src)

    # ============ prologue: load pre-transposed inputs, project ============
    with tc.tile_pool(name="ph0", bufs=1) as ph0, \
         tc.tile_pool(name="pps", bufs=1, space="PSUM") as pps:

        # kT first (longest dependency chain: kT -> K/V proj -> attention)
        k_T = ph0.tile([P, DO, NK], BF16, name="k_T")
        for c0 in range(0, NK, NK // 2):
            nc.sync.dma_start(
                k_T[:, :, c0:c0 + NK // 2],
                io["kT"][:, c0:c0 + NK // 2].rearrange("(o p) t -> p o t", p=P))
        wT = {}
        for n in ("WkT", "WvT", "WqT", "WoT"):
            t = ph0.tile([P, DO, D], BF16, name=n) if n != "WoT" else woT
            nc.sync.dma_start(t, io[n][:].rearrange("(o p) f -> p o f", p=P))
            wT[n] = t
        q_T = ph0.tile([P, DO, NQ], BF16, name="q_T")
        nc.sync.dma_start(q_T, io["qT"][:].rearrange("(o p) t -> p o t", p=P))

        # K/Q projections -> bf16 transposed layout, bias via K=1 matmul
        def project(w, src, b_row, dst, N):
            for o in range(DO):
                for c0 in range(0, N, 512):
                    ps = pps.tile([P, 512], F32, tag=f"pj{(c0 // 512) % 2}",
                                  name="pj")
                    for ki in range(DO):
                        nc.tensor.matmul(
                            ps, lhsT=w[:, ki, o * P:(o + 1) * P],
                            rhs=src[:, ki, c0:c0 + 512],
                            start=(ki == 0), stop=False)
                    nc.tensor.matmul(
                        ps, lhsT=b_row[0:1, o * P:(o + 1) * P],
                        rhs=ones_r[0:1, :], start=False, stop=True)
                    drain(dst[:, o, c0:c0 + 512], ps)

        project(wT["WkT"], k_T, bk_row, K_bf, NK)
        project(wT["WqT"], q_T, bq_row, Q_bf, NQ)

        # V projection: natural layout [tok, dv] per key tile
        for t in range(KT):
            ps = pps.tile([P, D], F32, tag=f"vp{t % 2}", name="vp")
            for ki in range(DO):
                nc.tensor.matmul(
                    ps, lhsT=k_T[:, ki, t * P:(t + 1) * P],
                    rhs=wT["WvT"][:, ki, :], start=(ki == 0), stop=False)
            nc.tensor.matmul(ps, lhsT=ones_r[0:1, :P], rhs=bv_row[0:1, :],
                             start=False, stop=True)
            drain(V_nat[:, t, :, :].rearrange("p h w -> p (h w)"), ps)

    # =================== attention ===================
    exp_split = EngineSplit(ACT_EXP_SHARE)
    with tc.tile_pool(name="attn_sb", bufs=1) as asb, \
         tc.tile_pool(name="sps", bufs=1, space="PSUM") as sps, \
         tc.tile_pool(name="ops", bufs=1, space="PSUM") as ops, \
         tc.tile_pool(name="nrm", bufs=2) as nrm:
        attn01 = asb.tile([P, KT, 2, SC], BF16, name="attn01")
        attn23 = asb.tile([P, KT, 2, SC], BF16, name="attn23")

        for qc in range(cfg.QCN):
            qsl = slice(qc * SC, (qc + 1) * SC)
            for o in range(DO):
                po = ops.tile([P, SC], F32, tag="po", name="po")
                pd = ops.tile([P, SC], F32, tag="pd", name="pd")

                def pv_step(t, po=po, pd=pd, o=o):
                    for m in range(4):
                        at = attn01 if m < 2 else attn23
                        rhs = at[:, t, m % 2, :]
                        nc.tensor.matmul(
                            po[32 * m:32 * m + 32, :],
                            lhsT=V_nat[:, t, o * 4 + m, :], rhs=rhs,
                            start=(t == 0), stop=(t == KT - 1),
                            tile_position=(0, 32 * m), skip_group_check=True)
                    for m in range(4):
                        at = attn01 if m < 2 else attn23
                        rhs = at[:, t, m % 2, :]
                        nc.tensor.matmul(
                            pd[32 * m:32 * m + 32, :], lhsT=ones32, rhs=rhs,
                            start=(t == 0), stop=(t == KT - 1),
                            tile_position=(0, 32 * m), skip_group_check=True)

                # scores + exp, with PV interleaved at lag 1
                for kt in range(KT):
                    ksl = slice(kt * P, (kt + 1) * P)
                    pj = (2 * kt) % 3
                    pa = sps.tile([P, 2, SC], F32, tag=f"sp{pj}", name="pa")
                    pb = sps.tile([P, 2, SC], F32, tag=f"sp{(pj + 1) % 3}",
                                  name="pb")
                    for m, (pt, sl) in enumerate(
                            ((pa, 0), (pa, 1), (pb, 0), (pb, 1))):
                        nc.tensor.matmul(
                            pt[:, sl, :],
                            lhsT=K_bf[32 * m:32 * m + 32, o, ksl],
                            rhs=Q_bf[32 * m:32 * m + 32, o, qsl],
                            start=True, stop=True,
                            tile_position=(32 * m, 0))
                    for at, pt in ((attn01, pa), (attn23, pb)):
                        if exp_split.take():
                            nc.scalar.activation(at[:, kt, :, :], pt, AF.Exp,
                                                 scale=1.0 / 16.0)
                        else:
                            nc.vector.tensor_scalar(
                                at[:, kt, :, :].bitcast(I16), pt,
                                EXP_A, EXP_B, OP.mult, OP.add)
                    if kt >= 1:
                        pv_step(kt - 1)
                pv_step(KT - 1)

                rec = nrm.tile([P, SC], F32, tag="rec", name="rec")
                nc.vector.reciprocal_approx_fast(rec, pd)
                osl = O_bf[:, o, qsl]
                nc.vector.tensor_tensor(osl, po, rec, OP.mult)
                nc.vector.tensor_tensor(osl, osl, Q_bf[:, o, qsl], OP.add)

    # =================== epilogue: LN0, FFN, LN1, out ===================
    # stage-interleaved across the two LC chunks so serial chains overlap
    with tc.tile_pool(name="ep_sb", bufs=1) as esb, \
         tc.tile_pool(name="st_ps", bufs=1, space="PSUM") as stp, \
         tc.tile_pool(name="bc_ps", bufs=1, space="PSUM") as bcp, \
         tc.tile_pool(name="f_ps", bufs=1, space="PSUM") as fps, \
         tc.tile_pool(name="o_ps", bufs=1, space="PSUM") as otp:
        NC = LC // P  # 4 query-pieces per chunk
        csls = [slice(c * LC, (c + 1) * LC) for c in range(cfg.LCN)]

        def ln_stats(src, csl, tag):
            """Stage 1: x^2 + ones-matmul stats -> st_sb [1, 2, LC] bf16."""
            x2 = esb.tile([P, DO, LC], BF16, tag=f"x2{tag}", name="x2")
            for o in range(DO):
                nc.scalar.activation(x2[:, o, :], src[:, o, csl], AF.Square)
            st_sb = esb.tile([1, 2, LC], BF16, tag=f"stsb{tag}", name="st_sb")
            for i in range(2):
                st = stp.tile([1, LC], F32, tag="st", name="st")
                for o in range(DO):
                    rhs = src[:, o, csl] if i == 0 else x2[:, o, :]
                    nc.tensor.matmul(st, lhsT=ones_k, rhs=rhs,
                                     start=(o == 0), stop=(o == DO - 1))
                nc.scalar.copy(st_sb[:, i, :], st)
            return st_sb

        def ln_rows(st_sb, tag, bf_out=False):
            """Stage 2: transpose stats pieces, row math -> A8/B8 [128, NC]."""
            st_t = stp.tile([P, 2 * NC], F32, tag="stt", name="st_t")
            for i in range(2):
                for j in range(NC):
                    nc.tensor.matmul(
                        st_t[:, i * NC + j:i * NC + j + 1],
                        lhsT=st_sb[0:1, i, j * P:(j + 1) * P],
                        rhs=ones_r[0:1, 0:1], start=True, stop=True)
            stt_sb = esb.tile([P, 2 * NC], F32, tag=f"sttsb{tag}", name="stt_sb")
            nc.vector.tensor_copy(stt_sb, st_t)
            sx, sx2 = stt_sb[:, 0:NC], stt_sb[:, NC:2 * NC]
            r8 = esb.tile([P, 5, NC], F32, tag=f"r8{tag}", name="r8")
            mu, ve, var_e, y2t, A8 = (r8[:, i, :] for i in range(5))
            nc.vector.tensor_scalar(mu, sx, 1.0 / D, None, OP.mult)
            nc.vector.tensor_scalar(ve, sx2, 1.0 / D, EPS, OP.mult, OP.add)
            nc.vector.tensor_tensor(var_e, mu, mu, OP.mult)
            nc.vector.tensor_tensor(var_e, ve, var_e, OP.subtract)
            y0 = esb.tile([P, NC], I16, tag=f"y0{tag}", name="y0")
            nc.vector.tensor_scalar(y0, var_e[:].bitcast(I32),
                                    RS_A, RS_B, OP.mult, OP.add)
            y0b = y0[:].bitcast(BF16)  # ~= rstd seed (+-3.7%)
            nc.vector.tensor_tensor(y2t, y0b, y0b, OP.mult)
            nc.vector.tensor_tensor(y2t, y2t, var_e, OP.mult)
            nc.vector.tensor_scalar(y2t, y2t, -0.5, 1.5, OP.mult, OP.add)
            nc.vector.tensor_tensor(A8, y2t, y0b, OP.mult)  # rstd
            B8 = r8[:, 3, :]  # reuse y2t slot
            nc.vector.scalar_tensor_tensor(B8, mu, -1.0, A8, OP.mult, OP.mult)
            if not bf_out:
                return A8, B8
            ab = esb.tile([P, 2, NC], BF16, tag=f"ab{tag}", name="ab")
            nc.vector.tensor_copy(ab[:, 0, :], A8)
            nc.vector.tensor_copy(ab[:, 1, :], B8)
            return ab

        # ---- LN0 ----
        st0 = [ln_stats(O_bf, csls[c], f"a{c}") for c in range(cfg.LCN)]
        ab0 = [ln_rows(st0[c], f"a{c}", bf_out=True) for c in range(cfg.LCN)]
        pab_sb = []
        for c in range(cfg.LCN):
            # broadcast A/B to [P, LC]: bf16 transpose + K=1 ones outer
            abr_ps = stp.tile([1, 2 * NC, P], F32, tag="abr", name="abr_ps")
            for i in range(2):
                for j in range(NC):
                    nc.tensor.matmul(
                        abr_ps[:, i * NC + j, :], lhsT=ab0[c][:, i, j:j + 1],
                        rhs=ident_bf, start=True, stop=True)
            abr = esb.tile([1, 2 * NC, P], BF16, tag=f"abrs{c}", name="abr")
            nc.scalar.copy(abr, abr_ps)
            pab = bcp.tile([P, 2, LC], F32, tag="pab", name="pab")
            for i in range(2):
                for j in range(NC):
                    nc.tensor.matmul(
                        pab[:, i, j * P:(j + 1) * P], lhsT=ones_r[0:1, 0:P],
                        rhs=abr[0:1, i * NC + j, :], start=True, stop=True)
            psb = esb.tile([P, 2, LC], BF16, tag=f"psb{c}", name="psb")
            drain(psb, pab)
            pab_sb.append(psb)
        # ---- LN0 apply + FFN ----
        for c in range(cfg.LCN):
            csl, psb = csls[c], pab_sb[c]
            for o in range(DO):
                xsl = X0[:, o, csl]
                nc.vector.tensor_tensor(xsl, O_bf[:, o, csl], psb[:, 0, :],
                                        OP.mult)
                nc.vector.tensor_tensor(xsl, xsl, psb[:, 1, :], OP.add)
                if cfg.ln0_gb:
                    nc.vector.scalar_tensor_tensor(
                        xsl, xsl, g0_pm[:, o:o + 1],
                        b0_pm[:, o:o + 1].to_broadcast([P, LC]),
                        OP.mult, OP.add)
        for c in range(cfg.LCN):
            csl = csls[c]
            for o in range(DO):
                fp = fps.tile([P, LC], F32, tag="f", name="fp")
                for ki in range(DO):
                    nc.tensor.matmul(fp, lhsT=woT[:, ki, o * P:(o + 1) * P],
                                     rhs=X0[:, ki, csl],
                                     start=(ki == 0), stop=(ki == DO - 1))
                h = esb.tile([P, LC], BF16, tag=f"h{o}{c}", name="h")
                nc.scalar.activation(h, fp, AF.Relu, bias=bo_pm[:, o:o + 1])
                nc.vector.tensor_tensor(X1[:, o, csl], X0[:, o, csl], h, OP.add)
        # ---- LN1 + transpose out ----
        st1 = [ln_stats(X1, csls[c], f"b{c}") for c in range(cfg.LCN)]
        ab1 = [ln_rows(st1[c], f"b{c}") for c in range(cfg.LCN)]
        for c in range(cfg.LCN):
            A81, B81 = ab1[c]
            for j in range(NC):
                t = c * NC + j
                tp = otp.tile([P, 2, P], BF16, tag="ot", name="tp")
                for o in range(DO):
                    nc.tensor.transpose(tp[:, o, :],
                                        X1[:, o, t * P:(t + 1) * P], ident_bf)
                ov = out_nat[:, t, :].rearrange("p (o c2) -> p o c2", c2=P)
                nc.vector.tensor_scalar(ov, tp, A81[:, j:j + 1], B81[:, j:j + 1],
                                        OP.mult, OP.add)
                if cfg.ln1_gb:
                    ovf = out_nat[:, t, :]
                    nc.vector.tensor_tensor(ovf, ovf, g1_bc, OP.mult)
                    nc.vector.tensor_tensor(ovf, ovf, b1_bc, OP.add)
        nc.sync.dma_start(io["out"][:].rearrange("(t p) d -> p t d", p=P),
                          out_nat)


def build(cfg: Cfg) -> bass.Bass:
    nc = bacc.Bacc("TRN2")
    io = {}
    for name, shape, dt in (
        ("qT", [cfg.D, cfg.NQ], BF16), ("kT", [cfg.D, cfg.NK], BF16),
        ("WqT", [cfg.D, cfg.D], BF16), ("WkT", [cfg.D, cfg.D], BF16),
        ("WvT", [cfg.D, cfg.D], BF16), ("WoT", [cfg.D, cfg.D], BF16),
        ("bq", [cfg.D], F32), ("bk", [cfg.D], F32), ("bv", [cfg.D], F32),
        ("bo", [cfg.D], F32), ("g0", [cfg.D], F32), ("b0", [cfg.D], F32),
        ("g1", [cfg.D], F32), ("b1", [cfg.D], F32),
    ):
        io[name] = nc.dram_tensor(name, shape, dt, kind="ExternalInput")
    io["out"] = nc.dram_tensor("out", [cfg.NQ, cfg.D], F32, kind="ExternalOutput")

    with tile.TileContext(nc) as tc:
        with ExitStack() as ctx:
            _emit(nc, tc, ctx, io, cfg)
    nc.compile()
    return nc


_CACHE = {}


def _get_nc(key, cfg):
    if key not in _CACHE:
        _CACHE[key] = build(cfg)
    return _CACHE[key]


def kernel(q, k, Wq, bq, Wk, bk, Wv, bv, Wo, bo, g0, b0, g1, b1, _trace=False):
    from concourse.bass_utils import run_bass_kernel_spmd

    B, Nq, D = q.shape
    Nk = k.shape[1]
    n_cores = 8
    halves = n_cores // B
    nq_c = Nq // halves
    ln0_gb = not (np.allclose(g0, 1.0) and np.allclose(b0, 0.0))
    ln1_gb = not (np.allclose(g1, 1.0) and np.allclose(b1, 0.0))
    cfg = Cfg(NQ=nq_c, NK=Nk, D=D, ln0_gb=ln0_gb, ln1_gb=ln1_gb)
    nc = _get_nc((nq_c, Nk, D, ln0_gb, ln1_gb), cfg)

    bf = ml_dtypes.bfloat16

    def t_bf(a):  # [N, D] fp32 -> [D, N] bf16 contiguous
        return np.ascontiguousarray(np.asarray(a, np.float32).T.astype(bf))

    shared = dict(WqT=t_bf(Wq), WkT=t_bf(Wk), WvT=t_bf(Wv), WoT=t_bf(Wo))
    for n, v in (("bq", bq), ("bk", bk), ("bv", bv), ("bo", bo),
                 ("g0", g0), ("b0", b0), ("g1", g1), ("b1", b1)):
        shared[n] = np.ascontiguousarray(v, dtype=np.float32)
    kT = [t_bf(k[b]) for b in range(B)]
    in_maps = []
    for c in range(n_cores):
        b, hf = c // halves, c % halves
        m = dict(shared)
        m["qT"] = t_bf(q[b, hf * nq_c:(hf + 1) * nq_c])
        m["kT"] = kT[b]
        in_maps.append(m)

    res = run_bass_kernel_spmd(nc, in_maps, core_ids=list(range(n_cores)),
                               trace=_trace)
    out = np.empty((B, Nq, D), np.float32)
    for c in range(n_cores):
        b, hf = c // halves, c % halves
        out[b, hf * nq_c:(hf + 1) * nq_c] = res.results[c]["out"]
    if _trace:
        return out, res
    return out


# revision 17
# speedup vs baseline: 2.7416x; 1.2535x over previous
"""MAB (Set-Transformer multihead attention block) Trainium2 Bass kernel, v4.

Reference math (fp32):
  Q = q @ Wq.T + bq ; K = k @ Wk.T + bk ; V = k @ Wv.T + bv    [B,N,256]
  per head h (8 heads x 32): s = Qh @ Kh.T / 16 ; a = softmax(s)
  Oh = Qh + a @ Vh ; o = concat(Oh) ; o = LN0(o) ; o = o + relu(o @ Wo.T + bo)
  out = LN1(o)

Sharding: 8 cores = (batch b in 0..3, query-half in 0..1); no collectives.

v4 vs v3:
  - q/k/W are pre-transposed and pre-cast to bf16 on the host (input
    layout prep in the kernel() wrapper): no on-device input transposes
    or casts, and half the DMA bytes. Device inputs are qT/kT/W*T bf16.
  - epilogue rewritten: every matmul operand bf16 (no more fp32
    LOW_HIGH two-pass matmuls anywhere), LN0 broadcast tensors drained
    to SBUF bf16 so the apply runs at DVE 2x, and the two LN/FFN chunks
    are emitted stage-interleaved so their serial chains overlap.
  - attention unchanged from v3: 4-way row-tiled concurrent score
    matmuls, PV/denominator col-tiled matmuls interleaved at lag 1,
    exp split ACT (table exp) / DVE (bf16 bit-trick), denominators via
    ones matmuls, 1/d via reciprocal_approx_fast, rstd via rsqrt
    bit-trick + Newton (single ACT table load in the whole kernel).
"""

import os
import sys
from contextlib import ExitStack

import numpy as np

for _p in ("/opt/trn_rl_repo", "/root/.axon_site/_ro/trn_rl_repo"):
    if os.path.isdir(_p) and _p not in sys.path:
        sys.path.insert(0, _p)

import ml_dtypes  # noqa: E402
import concourse.bass as bass  # noqa: E402
import concourse.tile as tile  # noqa: E402
from concourse import bacc, mybir  # noqa: E402
from concourse.masks import make_identity  # noqa: E402

F32 = mybir.dt.float32
BF16 = mybir.dt.bfloat16
I16 = mybir.dt.int16
I32 = mybir.dt.int32
P = 128
EPS = 1e-5

AF = mybir.ActivationFunctionType
OP = mybir.AluOpType

LOG2E = 1.4426950408889634
# exp(x/16) ~= bf16(bits = round(x*EXP_A + EXP_B))
EXP_A = 128.0 * LOG2E / 16.0
EXP_B = 128.0 * (127.0 - 0.04380)
# rsqrt(v) ~= bf16(bits = round(bits32(v)*RS_A + RS_B)), then 1 Newton step
RS_A = -64.0 / (1 << 23)
RS_B = 24375.283445

ACT_EXP_SHARE = 0.56  # fraction of exp drains on ScalarE


class Cfg:
    def __init__(self, NQ=1024, NK=2048, D=256, H=8, ln0_gb=False, ln1_gb=False):
        self.NQ, self.NK, self.D, self.H = NQ, NK, D, H
        self.HD = D // H            # 32
        self.DO = D // P            # 2
        self.QT = NQ // P           # 8
        self.KT = NK // P           # 16
        self.SC = 512               # score/attn q-chunk
        self.QCN = NQ // self.SC    # 2
        self.LC = 512               # ln/ffn q-chunk
        self.LCN = NQ // self.LC
        self.ln0_gb = ln0_gb        # apply non-trivial g0/b0
        self.ln1_gb = ln1_gb
        assert self.HD == 32 and self.DO == 2


class EngineSplit:
    """Bresenham-style assigner: returns True for ACT with given share."""

    def __init__(self, share):
        self.share = share
        self.acc = 0.0

    def take(self):
        self.acc += self.share
        if self.acc >= 1.0:
            self.acc -= 1.0
            return True
        return False


def _emit(nc: bass.Bass, tc: tile.TileContext, ctx: ExitStack, io: dict, cfg: Cfg):
    NQ, NK, D, H = cfg.NQ, cfg.NK, cfg.D, cfg.H
    DO, QT, KT, SC, LC = cfg.DO, cfg.QT, cfg.KT, cfg.SC, cfg.LC

    const = ctx.enter_context(tc.tile_pool(name="const", bufs=1))
    persist = ctx.enter_context(tc.tile_pool(name="persist", bufs=1))

    # ---- constants ----
    ident = const.tile([P, P], F32)
    make_identity(nc, ident)
    ident_bf = const.tile([P, P], BF16)
    nc.vector.tensor_copy(ident_bf, ident)
    ones32 = const.tile([P, 32], BF16)
    nc.vector.memset(ones32, 1.0)
    ones_r = const.tile([1, 512], BF16)  # ones row: bias-matmul rhs / lhsT
    nc.vector.memset(ones_r, 1.0)
    ones_k = const.tile([P, 1], BF16)    # LN stats lhsT (partition sum)
    nc.vector.memset(ones_k, 1.0)

    def vec_row(name, dtype=BF16):
        tf = const.tile([1, D], F32, name=f"{name}_rowf")
        nc.sync.dma_start(tf, io[name][:].rearrange("(o d) -> o d", o=1))
        if dtype == F32:
            return tf
        t = const.tile([1, D], BF16, name=f"{name}_row")
        nc.vector.tensor_copy(t, tf)
        return t

    bq_row, bk_row, bv_row = vec_row("bq"), vec_row("bk"), vec_row("bv")

    def vec_pm(name):
        t = const.tile([P, DO], F32, name=f"{name}_pm")
        nc.sync.dma_start(t, io[name][:].rearrange("(o p) -> p o", p=P))
        return t

    bo_pm = vec_pm("bo")
    g0_pm = vec_pm("g0") if cfg.ln0_gb else None
    b0_pm = vec_pm("b0") if cfg.ln0_gb else None
    if cfg.ln1_gb:
        g1_bc = const.tile([P, D], F32)
        nc.gpsimd.partition_broadcast(g1_bc, vec_row("g1", F32))
        b1_bc = const.tile([P, D], F32)
        nc.gpsimd.partition_broadcast(b1_bc, vec_row("b1", F32))

    # ---- persistent tensors (all bf16) ----
    Q_bf = persist.tile([P, DO, NQ], BF16, name="Q_bf")
    K_bf = persist.tile([P, DO, NK], BF16, name="K_bf")
    V_nat = persist.tile([P, KT, H, 32], BF16, name="V_nat")  # [tok, dv]/tile
    woT = persist.tile([P, DO, D], BF16, name="woT")
    O_bf = persist.tile([P, DO, NQ], BF16, name="O_bf")
    X0 = persist.tile([P, DO, NQ], BF16, name="X0")
    X1 = persist.tile([P, DO, NQ], BF16, name="X1")
    out_nat = persist.tile([P, QT, D], F32, name="out_nat")

    drain_split = EngineSplit(0.5)

    def drain(dst, src):
        if drain_split.take():
            nc.scalar.copy(dst, src)
        else:
            nc.vector.tensor_copy(dst, src)

    # ============ prologue: load pre-transposed inputs ============
    ph0 = ctx.enter_context(tc.tile_pool(name="ph0", bufs=1))

    # kT first (longest dependency chain: kT -> K/V proj -> attention)
    k_T = ph0.tile([P, DO, NK], BF16, name="k_T")
    for c0 in range(0, NK, NK // 2):
        nc.sync.dma_start(
            k_T[:, :, c0:c0 + NK // 2],
            io["kT"][:, c0:c0 + NK // 2].rearrange("(o p) t -> p o t", p=P))
    wT = {}
    for n in ("WkT", "WvT", "WqT", "WoT"):
        t = ph0.tile([P, DO, D], BF16, name=n) if n != "WoT" else woT
        nc.sync.dma_start(t, io[n][:].rearrange("(o p) f -> p o f", p=P))
        wT[n] = t
    q_T = ph0.tile([P, DO, NQ], BF16, name="q_T")
    nc.sync.dma_start(q_T, io["qT"][:].rearrange("(o p) t -> p o t", p=P))

    def proj_group(w, src, b_row, dst, o, c0, ps):
        for ki in range(DO):
            nc.tensor.matmul(
                ps, lhsT=w[:, ki, o * P:(o + 1) * P],
                rhs=src[:, ki, c0:c0 + 512], start=(ki == 0), stop=False)
        nc.tensor.matmul(ps, lhsT=b_row[0:1, o * P:(o + 1) * P],
                         rhs=ones_r[0:1, :], start=False, stop=True)
        drain(dst[:, o, c0:c0 + 512], ps)

    # o=0 projections of K and Q gate the first score matmuls: do them
    # in a short serial pre-phase; everything else rides inside block 0.
    with tc.tile_pool(name="pps", bufs=1, space="PSUM") as pps:
        for c in range(4):
            ps = pps.tile([P, 512], F32, tag=f"pj{c % 2}", name="pj")
            proj_group(wT["WkT"], k_T, bk_row, K_bf, 0, c * 512, ps)
        for c in range(2):
            ps = pps.tile([P, 512], F32, tag=f"pj{c % 2}", name="pj")
            proj_group(wT["WqT"], q_T, bq_row, Q_bf, 0, c * 512, ps)

    # =================== attention ===================
    exp_split = EngineSplit(ACT_EXP_SHARE)
    with tc.tile_pool(name="attn_sb", bufs=1) as asb, \
         tc.tile_pool(name="sps", bufs=1, space="PSUM") as sps, \
         tc.tile_pool(name="ops", bufs=1, space="PSUM") as ops, \
         tc.tile_pool(name="nrm", bufs=2) as nrm:
        attn01 = asb.tile([P, KT, 2, SC], BF16, name="attn01")
        attn23 = asb.tile([P, KT, 2, SC], BF16, name="attn23")

        # leftover projections, emitted one group per kt-step of block 0
        # (they use the sp2 score buffers, which block 0 leaves free)
        extras = []
        for t in range(KT):
            def vproj(t=t):
                ps = sps.tile([P, 2, SC], F32, tag="sp2", name="pvv")
                psv = ps[:, 0, 0:D]
                for ki in range(DO):
                    nc.tensor.matmul(
                        psv, lhsT=k_T[:, ki, t * P:(t + 1) * P],
                        rhs=wT["WvT"][:, ki, :], start=(ki == 0), stop=False)
                nc.tensor.matmul(psv, lhsT=ones_r[0:1, :P],
                                 rhs=bv_row[0:1, :], start=False, stop=True)
                drain(V_nat[:, t, :, :].rearrange("p h w -> p (h w)"), psv)
            extras.append(vproj)
        for c in range(4):
            def kproj(c=c):
                ps = sps.tile([P, 2, SC], F32, tag="sp2", name="pvk")
                proj_group(wT["WkT"], k_T, bk_row, K_bf, 1, c * 512,
                           ps[:, 1, :])
            extras.append(kproj)
        for c in range(2):
            def qproj(c=c):
                ps = sps.tile([P, 2, SC], F32, tag="sp2", name="pvq")
                proj_group(wT["WqT"], q_T, bq_row, Q_bf, 1, c * 512,
                           ps[:, 1, :])
            extras.append(qproj)

        for qc in range(cfg.QCN):
            qsl = slice(qc * SC, (qc + 1) * SC)
            for o in range(DO):
                blk0 = qc == 0 and o == 0
                po = ops.tile([P, SC], F32, tag="po", name="po")
                pd = ops.tile([P, SC], F32, tag="pd", name="pd")

                def pv_step(t, po=po, pd=pd, o=o):
                    for m in range(4):
                        at = attn01 if m < 2 else attn23
                        rhs = at[:, t, m % 2, :]
                        nc.tensor.matmul(
                            po[32 * m:32 * m + 32, :],
                            lhsT=V_nat[:, t, o * 4 + m, :], rhs=rhs,
                            start=(t == 0), stop=(t == KT - 1),
                            tile_position=(0, 32 * m), skip_group_check=True)
                    for m in range(4):
                        at = attn01 if m < 2 else attn23
                        rhs = at[:, t, m % 2, :]
                        nc.tensor.matmul(
                            pd[32 * m:32 * m + 32, :], lhsT=ones32, rhs=rhs,
                            start=(t == 0), stop=(t == KT - 1),
                            tile_position=(0, 32 * m), skip_group_check=True)

                # scores + exp, with PV interleaved at lag 1
                for kt in range(KT):
                    if blk0:
                        extras.pop(0)()          # V tile kt
                        if extras and kt >= KT - 7:
                            extras.pop(0)()      # K/Q o=1 projections
                    ksl = slice(kt * P, (kt + 1) * P)
                    pj = (2 * kt) % (2 if blk0 else 3)
                    pa = sps.tile([P, 2, SC], F32, tag=f"sp{pj}", name="pa")
                    pb = sps.tile([P, 2, SC], F32,
                                  tag=f"sp{(pj + 1) % (2 if blk0 else 3)}",
                                  name="pb")
                    for m, (pt, sl) in enumerate(
                            ((pa, 0), (pa, 1), (pb, 0), (pb, 1))):
                        nc.tensor.matmul(
                            pt[:, sl, :],
                            lhsT=K_bf[32 * m:32 * m + 32, o, ksl],
                            rhs=Q_bf[32 * m:32 * m + 32, o, qsl],
                            start=True, stop=True,
                            tile_position=(32 * m, 0))
                    for at, pt in ((attn01, pa), (attn23, pb)):
                        if exp_split.take():
                            nc.scalar.activation(at[:, kt, :, :], pt, AF.Exp,
                                                 scale=1.0 / 16.0)
                        else:
                            nc.vector.tensor_scalar(
                                at[:, kt, :, :].bitcast(I16), pt,
                                EXP_A, EXP_B, OP.mult, OP.add)
                    if kt >= 1:
                        pv_step(kt - 1)
                pv_step(KT - 1)

                rec = nrm.tile([P, SC], F32, tag="rec", name="rec")
                nc.vector.reciprocal_approx_fast(rec, pd)
                osl = O_bf[:, o, qsl]
                nc.vector.tensor_tensor(osl, po, rec, OP.mult)
                nc.vector.tensor_tensor(osl, osl, Q_bf[:, o, qsl], OP.add)

    # =================== epilogue: LN0, FFN, LN1, out ===================
    # stage-interleaved across the two LC chunks so serial chains overlap
    with tc.tile_pool(name="ep_sb", bufs=1) as esb, \
         tc.tile_pool(name="st_ps", bufs=1, space="PSUM") as stp, \
         tc.tile_pool(name="bc_ps", bufs=1, space="PSUM") as bcp, \
         tc.tile_pool(name="f_ps", bufs=1, space="PSUM") as fps, \
         tc.tile_pool(name="o_ps", bufs=1, space="PSUM") as otp:
        NC = LC // P  # 4 query-pieces per chunk
        csls = [slice(c * LC, (c + 1) * LC) for c in range(cfg.LCN)]

        def ln_stats(src, csl, tag):
            """Stage 1: x^2 + ones-matmul stats -> st_sb [1, 2, LC] bf16."""
            x2 = esb.tile([P, DO, LC], BF16, tag=f"x2{tag}", name="x2")
            for o in range(DO):
                nc.scalar.activation(x2[:, o, :], src[:, o, csl], AF.Square)
            st_sb = esb.tile([1, 2, LC], BF16, tag=f"stsb{tag}", name="st_sb")
            for i in range(2):
                st = stp.tile([1, LC], F32, tag="st", name="st")
                for o in range(DO):
                    rhs = src[:, o, csl] if i == 0 else x2[:, o, :]
                    nc.tensor.matmul(st, lhsT=ones_k, rhs=rhs,
                                     start=(o == 0), stop=(o == DO - 1))
                nc.scalar.copy(st_sb[:, i, :], st)
            return st_sb

        def ln_rows(st_sb, tag, bf_out=False):
            """Stage 2: transpose stats pieces, row math -> A8/B8 [128, NC]."""
            st_t = stp.tile([P, 2 * NC], F32, tag="stt", name="st_t")
            for i in range(2):
                for j in range(NC):
                    nc.tensor.matmul(
                        st_t[:, i * NC + j:i * NC + j + 1],
                        lhsT=st_sb[0:1, i, j * P:(j + 1) * P],
                        rhs=ones_r[0:1, 0:1], start=True, stop=True)
            stt_sb = esb.tile([P, 2 * NC], F32, tag=f"sttsb{tag}", name="stt_sb")
            nc.vector.tensor_copy(stt_sb, st_t)
            sx, sx2 = stt_sb[:, 0:NC], stt_sb[:, NC:2 * NC]
            r8 = esb.tile([P, 5, NC], F32, tag=f"r8{tag}", name="r8")
            mu, ve, var_e, y2t, A8 = (r8[:, i, :] for i in range(5))
            nc.vector.tensor_scalar(mu, sx, 1.0 / D, None, OP.mult)
            nc.vector.tensor_scalar(ve, sx2, 1.0 / D, EPS, OP.mult, OP.add)
            nc.vector.tensor_tensor(var_e, mu, mu, OP.mult)
            nc.vector.tensor_tensor(var_e, ve, var_e, OP.subtract)
            y0 = esb.tile([P, NC], I16, tag=f"y0{tag}", name="y0")
            nc.vector.tensor_scalar(y0, var_e[:].bitcast(I32),
                                    RS_A, RS_B, OP.mult, OP.add)
            y0b = y0[:].bitcast(BF16)  # ~= rstd seed (+-3.7%)
            nc.vector.tensor_tensor(y2t, y0b, y0b, OP.mult)
            nc.vector.tensor_tensor(y2t, y2t, var_e, OP.mult)
            nc.vector.tensor_scalar(y2t, y2t, -0.5, 1.5, OP.mult, OP.add)
            nc.vector.tensor_tensor(A8, y2t, y0b, OP.mult)  # rstd
            B8 = r8[:, 3, :]  # reuse y2t slot
            nc.vector.scalar_tensor_tensor(B8, mu, -1.0, A8, OP.mult, OP.mult)
            if not bf_out:
                return A8, B8
            ab = esb.tile([P, 2, NC], BF16, tag=f"ab{tag}", name="ab")
            nc.vector.tensor_copy(ab[:, 0, :], A8)
            nc.vector.tensor_copy(ab[:, 1, :], B8)
            return ab

        # ---- LN0 ----
        st0 = [ln_stats(O_bf, csls[c], f"a{c}") for c in range(cfg.LCN)]
        ab0 = [ln_rows(st0[c], f"a{c}", bf_out=True) for c in range(cfg.LCN)]
        pab_sb = []
        for c in range(cfg.LCN):
            # broadcast A/B to [P, LC]: bf16 transpose + K=1 ones outer
            abr_ps = stp.tile([1, 2 * NC, P], F32, tag="abr", name="abr_ps")
            for i in range(2):
                for j in range(NC):
                    nc.tensor.matmul(
                        abr_ps[:, i * NC + j, :], lhsT=ab0[c][:, i, j:j + 1],
                        rhs=ident_bf, start=True, stop=True)
            abr = esb.tile([1, 2 * NC, P], BF16, tag=f"abrs{c}", name="abr")
            nc.scalar.copy(abr, abr_ps)
            pab = bcp.tile([P, 2, LC], F32, tag="pab", name="pab")
            for i in range(2):
                for j in range(NC):
                    nc.tensor.matmul(
                        pab[:, i, j * P:(j + 1) * P], lhsT=ones_r[0:1, 0:P],
                        rhs=abr[0:1, i * NC + j, :], start=True, stop=True)
            psb = esb.tile([P, 2, LC], BF16, tag=f"psb{c}", name="psb")
            drain(psb, pab)
            pab_sb.append(psb)
        # ---- LN0 apply + FFN ----
        for c in range(cfg.LCN):
            csl, psb = csls[c], pab_sb[c]
            for o in range(DO):
                xsl = X0[:, o, csl]
                nc.vector.tensor_tensor(xsl, O_bf[:, o, csl], psb[:, 0, :],
                                        OP.mult)
                nc.vector.tensor_tensor(xsl, xsl, psb[:, 1, :], OP.add)
                if cfg.ln0_gb:
                    nc.vector.scalar_tensor_tensor(
                        xsl, xsl, g0_pm[:, o:o + 1],
                        b0_pm[:, o:o + 1].to_broadcast([P, LC]),
                        OP.mult, OP.add)
        for c in range(cfg.LCN):
            csl = csls[c]
            for o in range(DO):
                fp = fps.tile([P, LC], F32, tag="f", name="fp")
                for ki in range(DO):
                    nc.tensor.matmul(fp, lhsT=woT[:, ki, o * P:(o + 1) * P],
                                     rhs=X0[:, ki, csl],
                                     start=(ki == 0), stop=(ki == DO - 1))
                h = esb.tile([P, LC], BF16, tag=f"h{o}{c}", name="h")
                nc.scalar.activation(h, fp, AF.Relu, bias=bo_pm[:, o:o + 1])
                nc.vector.tensor_tensor(X1[:, o, csl], X0[:, o, csl], h, OP.add)
        # ---- LN1 + transpose out ----
        st1 = [ln_stats(X1, csls[c], f"b{c}") for c in range(cfg.LCN)]
        ab1 = [ln_rows(st1[c], f"b{c}") for c in range(cfg.LCN)]
        for c in range(cfg.LCN):
            A81, B81 = ab1[c]
            for j in range(NC):
                t = c * NC + j
                tp = otp.tile([P, 2, P], BF16, tag="ot", name="tp")
                for o in range(DO):
                    nc.tensor.transpose(tp[:, o, :],
                                        X1[:, o, t * P:(t + 1) * P], ident_bf)
                ov = out_nat[:, t, :].rearrange("p (o c2) -> p o c2", c2=P)
                nc.vector.tensor_scalar(ov, tp, A81[:, j:j + 1], B81[:, j:j + 1],
                                        OP.mult, OP.add)
                if cfg.ln1_gb:
                    ovf = out_nat[:, t, :]
                    nc.vector.tensor_tensor(ovf, ovf, g1_bc, OP.mult)
                    nc.vector.tensor_tensor(ovf, ovf, b1_bc, OP.add)
            nc.sync.dma_start(
                io["out"][csls[c], :].rearrange("(t p) d -> p t d", p=P),
                out_nat[:, c * NC:(c + 1) * NC, :])


def build(cfg: Cfg) -> bass.Bass:
    nc = bacc.Bacc("TRN2")
    io = {}
    for name, shape, dt in (
        ("qT", [cfg.D, cfg.NQ], BF16), ("kT", [cfg.D, cfg.NK], BF16),
        ("WqT", [cfg.D, cfg.D], BF16), ("WkT", [cfg.D, cfg.D], BF16),
        ("WvT", [cfg.D, cfg.D], BF16), ("WoT", [cfg.D, cfg.D], BF16),
        ("bq", [cfg.D], F32), ("bk", [cfg.D], F32), ("bv", [cfg.D], F32),
        ("bo", [cfg.D], F32), ("g0", [cfg.D], F32), ("b0", [cfg.D], F32),
        ("g1", [cfg.D], F32), ("b1", [cfg.D], F32),
    ):
        io[name] = nc.dram_tensor(name, shape, dt, kind="ExternalInput")
    io["out"] = nc.dram_tensor("out", [cfg.NQ, cfg.D], F32, kind="ExternalOutput")

    with tile.TileContext(nc) as tc:
        with ExitStack() as ctx:
            _emit(nc, tc, ctx, io, cfg)
    nc.compile()
    return nc


_CACHE = {}


def _get_nc(key, cfg):
    if key not in _CACHE:
        _CACHE[key] = build(cfg)
    return _CACHE[key]


def kernel(q, k, Wq, bq, Wk, bk, Wv, bv, Wo, bo, g0, b0, g1, b1, _trace=False):
    from concourse.bass_utils import run_bass_kernel_spmd

    B, Nq, D = q.shape
    Nk = k.shape[1]
    n_cores = 8
    halves = n_cores // B
    nq_c = Nq // halves
    ln0_gb = not (np.allclose(g0, 1.0) and np.allclose(b0, 0.0))
    ln1_gb = not (np.allclose(g1, 1.0) and np.allclose(b1, 0.0))
    cfg = Cfg(NQ=nq_c, NK=Nk, D=D, ln0_gb=ln0_gb, ln1_gb=ln1_gb)
    nc = _get_nc((nq_c, Nk, D, ln0_gb, ln1_gb), cfg)

    bf = ml_dtypes.bfloat16

    def t_bf(a):  # [N, D] fp32 -> [D, N] bf16 contiguous
        return np.ascontiguousarray(np.asarray(a, np.float32).T.astype(bf))

    shared = dict(WqT=t_bf(Wq), WkT=t_bf(Wk), WvT=t_bf(Wv), WoT=t_bf(Wo))
    for n, v in (("bq", bq), ("bk", bk), ("bv", bv), ("bo", bo),
                 ("g0", g0), ("b0", b0), ("g1", g1), ("b1", b1)):
        shared[n] = np.ascontiguousarray(v, dtype=np.float32)
    kT = [t_bf(k[b]) for b in range(B)]
    in_maps = []
    for c in range(n_cores):
        b, hf = c // halves, c % halves
        m = dict(shared)
        m["qT"] = t_bf(q[b, hf * nq_c:(hf + 1) * nq_c])
        m["kT"] = kT[b]
        in_maps.append(m)

    res = run_bass_kernel_spmd(nc, in_maps, core_ids=list(range(n_cores)),
                               trace=_trace)
    out = np.empty((B, Nq, D), np.float32)
    for c in range(n_cores):
        b, hf = c // halves, c % halves
        out[b, hf * nq_c:(hf + 1) * nq_c] = res.results[c]["out"]
    if _trace:
        return out, res
    return out
